# revision 23
# baseline (speedup 1.0000x reference)
"""Trainium2 Bass kernel for a 2-layer GAT encoder (edge-softmax message passing).

Strategy (8 NeuronCores, SPMD single program):
- dst-node partition across cores; host packs each core's dst nodes into
  fixed-count "windows" (<=128 nodes each) and edges into fixed-count
  128-edge tiles per window (K_LO tiles for src in the low half of the
  padded node space, K_HI for the high half -- dma_gather indices are int16).
- Node phase: h_ext = x @ [W | W@att_src | W@att_dst] per core slice,
  fp16 row table written to DRAM in chunks, chunked AllGather pipelined
  with the node matmuls -> full table on every core.
- Edge phase per 128-edge tile: dma_gather rows by src; one-hot matrices
  (fp8, host-precomputed, streamed from DRAM); per-edge a_d via one-hot
  matmul; p = exp(leaky_relu(a_s+a_d)) on the Scalar engine (softmax shift
  invariance makes the segment-max subtraction unnecessary); segment-sum of
  [p*h | p] via one-hot matmul accumulated in PSUM per window; epilogue
  divides and applies bias/ELU. Layer-2 node matmul + table write is
  interleaved per-window into the layer-1 edge phase, with the second
  chunked AllGather riding on top.
- Output rows are window-padded; host de-permutes to the original node order.
"""

import numpy as np

NCORES = 8
HALF = 32768          # int16 gather index limit
ROW1 = 256            # fp16 elems per layer-1 table row (512B): h128|a_s4|a_d4|pad
ROW2 = 128            # fp16 elems per layer-2 table row (256B): h64|a_s|pad
H1, C1 = 4, 32
H2, C2 = 1, 64
IN_CH = 128
HC1 = H1 * C1         # 128
NEG_SLOPE = 0.2
EPS = 1e-16


# ---------------------------------------------------------------------------
# Host-side preprocessing
# ---------------------------------------------------------------------------

def _pack_windows(src, dst, n_nodes, k_lo, k_hi, boundary_arr, per_core,
                  _cache={}):
    """Greedy-pack each core's dst nodes into windows (<=128 nodes, <=k_lo
    lo-tiles, <=k_hi hi-tiles). lo/hi = src node below/above its own core's
    chunk boundary (boundary_arr[core]). Edges must be sorted by dst."""
    key = id(src)
    if _cache.get("key") != key:
        order = np.argsort(dst, kind="stable")
        src_s, dst_s = src[order], dst[order]
        counts = np.bincount(dst_s, minlength=n_nodes)
        starts = np.concatenate([[0], np.cumsum(counts)])
        _cache.update(key=key, src_s=src_s, dst_s=dst_s, starts=starts)
    src_s, dst_s, starts = _cache["src_s"], _cache["dst_s"], _cache["starts"]
    lo_m_s = src_s < boundary_arr[src_s // per_core]
    nlo_n = np.bincount(dst_s[lo_m_s], minlength=n_nodes)
    nall_n = np.bincount(dst_s, minlength=n_nodes)
    nhi_n = nall_n - nlo_n
    cap_lo, cap_hi = k_lo * 128, k_hi * 128
    cores = []
    for c in range(NCORES):
        lo_n, hi_n = c * per_core, (c + 1) * per_core
        bounds = []  # window node ranges [a, b)
        a = lo_n
        cl = ch = cn = 0
        for n in range(lo_n, hi_n):
            el, eh = int(nlo_n[n]), int(nhi_n[n])
            if cn >= 128 or cl + el > cap_lo or ch + eh > cap_hi:
                assert n > a, "single node exceeds tile budget"
                bounds.append((a, n))
                a, cl, ch, cn = n, 0, 0, 0
            cl += el; ch += eh; cn += 1
        bounds.append((a, hi_n))
        wins = []
        for (a, b) in bounds:
            e0, e1_ = starts[a], starts[b]
            es = src_s[e0:e1_]
            dr = (dst_s[e0:e1_] - a).astype(np.int32)
            lm = lo_m_s[e0:e1_]
            wins.append((a, b, es[lm], es[~lm], dr[lm], dr[~lm]))
        cores.append(wins)
    return cores


def _layout(cores, gw, n_nodes, per_core):
    """Window count, chunk split, and the two permutations.
    pi: node -> global chunk-major table row (chunk 0 = windows [0,W0) of
    every core, chunk 1 = the rest).  po: node -> core-local padded slot
    (c*P + w*128 + s), the output row order."""
    W = max(len(w) for w in cores)
    W = ((W + gw - 1) // gw) * gw
    W0 = ((W // 2 + 1) // 2) * 2  # even: chunk boundary on a group boundary
    P = W * 128
    CH0 = NCORES * W0 * 128
    pi = np.zeros(n_nodes, np.int64)
    po = np.zeros(n_nodes, np.int64)
    boundary_arr = np.full(NCORES, 0, np.int64)
    for c, wins in enumerate(cores):
        boundary_arr[c] = (c + 1) * per_core
        for w, (a, b, *_r) in enumerate(wins):
            ids = np.arange(a, b)
            po[ids] = c * P + w * 128 + (ids - a)
            if w < W0:
                pi[ids] = c * W0 * 128 + w * 128 + (ids - a)
            else:
                pi[ids] = CH0 + c * (W - W0) * 128 + (w - W0) * 128 + (ids - a)
            if w == W0:
                boundary_arr[c] = a
    return W, W0, P, CH0, pi, po, boundary_arr


def host_prep(x, edge_index, n_nodes, k_lo, k_hi, gw):
    """Build the permutations, per-core metadata and index arrays."""
    src = np.ascontiguousarray(edge_index[0]).astype(np.int64)
    dst = np.ascontiguousarray(edge_index[1]).astype(np.int64)
    per_core = n_nodes // NCORES
    assert per_core * NCORES == n_nodes

    # fixpoint on the per-core lo/hi chunk boundaries: the packer's mask only
    # shapes the tile budgets; final buckets use the true pi and re-verify.
    boundary_arr = np.array([c * per_core + per_core // 2 for c in range(NCORES)],
                            np.int64)
    ok = False
    for it in range(14):
        cores = _pack_windows(src, dst, n_nodes, k_lo, k_hi, boundary_arr, per_core)
        W, W0, P, CH0, pi, po, nb = _layout(cores, gw, n_nodes, per_core)
        CH1 = NCORES * (W - W0) * 128
        if CH0 < 32768 and CH1 < 32768:
            # accept when every window's TRUE-bucket counts fit the budgets
            ok = True
            for c, wins in enumerate(cores):
                for (_a, _b, lo_s, hi_s, _ld, _hd) in wins:
                    es_ = np.concatenate([lo_s, hi_s])
                    nlo = int((pi[es_] < CH0).sum())
                    if nlo > k_lo * 128 or (es_.shape[0] - nlo) > k_hi * 128:
                        ok = False
                        break
                if not ok:
                    break
            if ok:
                break
        boundary_arr = nb
    assert ok, "chunk-boundary fixpoint failed"
    assert P * NCORES <= 65536

    K = k_lo + k_hi
    meta = []
    G = W // gw
    for c, wins in enumerate(cores):
        idx_lo = np.zeros((W, k_lo * 128), np.int16)
        idx_hi = np.zeros((W, k_hi * 128), np.int16)
        drel = np.full((W, K, 128), 255, np.int32)  # [window, tile-in-window, slot]
        for w, (a, b, lo_s, hi_s, lo_dr, hi_dr) in enumerate(wins):
            es = np.concatenate([lo_s, hi_s])
            dr = np.concatenate([lo_dr, hi_dr])
            lm = pi[es] < CH0                      # true buckets
            pl = pi[es[lm]]
            ph = pi[es[~lm]] - CH0
            ld, hd = dr[lm], dr[~lm]
            assert (pl < CH0).all() and (ph >= 0).all() and (ph < CH1).all()
            idx_lo[w, :len(pl)] = pl.astype(np.int16)
            idx_hi[w, :len(ph)] = ph.astype(np.int16)
            dr_pad = np.full(k_lo * 128, 255, np.int32)
            dr_pad[:len(ld)] = ld
            drel[w, :k_lo] = dr_pad.reshape(k_lo, 128)
            dr_pad = np.full(k_hi * 128, 255, np.int32)
            dr_pad[:len(hd)] = hd
            drel[w, k_lo:] = dr_pad.reshape(k_hi, 128)
        # group-tile order: per group: [lo tiles of gw windows][hi tiles of gw windows]
        tile_order = []  # (window, tile-in-window-index)
        for g in range(G):
            for w in range(g * gw, (g + 1) * gw):
                tile_order += [(w, t) for t in range(k_lo)]
            for w in range(g * gw, (g + 1) * gw):
                tile_order += [(w, k_lo + t) for t in range(k_hi)]
        to = np.array(tile_order)
        drel_t = drel[to[:, 0], to[:, 1]]            # [W*K, 128]
        # one-hot matrices, fp8(e4m3) encoded as raw uint8 bit patterns:
        # 1.0 -> 0x38, 0.0 -> 0x00.
        j = np.arange(128, dtype=np.int32)
        # e1[p, tile, j] = (drel_t[tile, p] == j): edge-slot on partitions
        e1 = (drel_t.T[:, :, None] == j[None, None, :]).astype(np.uint8) * 0x38
        # sm[p, tile, e] = (p == drel_t[tile, e]): dst-rel on partitions
        sm = (j[:, None, None] == drel_t[None, :, :]).astype(np.uint8) * 0x38
        # merged one-hot stream: per group [e1 tiles | sm tiles], fp8 bytes
        T_ = gw * K
        e1g = e1.reshape(128, G, T_, 128)
        smg = sm.reshape(128, G, T_, 128)
        es_ = np.concatenate([e1g, smg], axis=2)         # [128, G, 2T, 128]
        # merged idx stream: per group [lo idxs | hi idxs], wrapped by 16
        def wrap16(a):
            # idx j lives at [j%16, j//16], replicated into all 8 Q7 core
            # partition groups (HW reads each group independently)
            return np.ascontiguousarray(np.tile(a.reshape(-1, 16).T, (8, 1)))
        nlo, nhi = gw * k_lo * 128, gw * k_hi * 128
        ilg = idx_lo.reshape(G, nlo)
        ihg = idx_hi.reshape(G, nhi)
        idx = np.concatenate([ilg, ihg], axis=1)          # [G, nlo+nhi]
        meta.append(dict(
            idx=wrap16(idx),
            e1=np.ascontiguousarray(e1.reshape(128, -1)),
            sm=np.ascontiguousarray(sm.reshape(128, -1)),
            es=np.ascontiguousarray(es_.reshape(128, -1)),
        ))
    return dict(cores=cores, pi=pi, po=po, W=W, W0=W0, P=P, K=K, k_lo=k_lo,
                k_hi=k_hi, gw=gw, meta=meta, n_nodes=n_nodes, per_core=per_core)


def pick_config(x, edge_index, n_nodes):
    """Try candidate (k_lo, k_hi) packings, return the prep with fewest tiles."""
    gw = 2
    E = edge_index.shape[1]
    lam = E / n_nodes * 128
    base = max(int(np.ceil(lam * 0.5 / 128)), 1)
    cands = []
    for dlo in (0, 1, -1, 2):
        for dhi in (0, 1, -1, 2):
            if base + dlo >= 1 and base + dhi >= 1:
                cands.append((base + dlo, base + dhi))
    cands.sort(key=lambda c: c[0] + c[1])
    best = None
    for k_lo, k_hi in cands:
        try:
            p = host_prep(x, edge_index, n_nodes, k_lo, k_hi, gw)
        except AssertionError:
            continue
        slots = p["W"] * p["K"]
        if best is None or slots < best["W"] * best["K"]:
            best = p
        if slots <= (E / NCORES) / 128 * 1.11:  # good enough
            break
    assert best is not None, "no feasible packing found"
    return best


# ---------------------------------------------------------------------------
# Bass program
# ---------------------------------------------------------------------------

def build_program(cfg):
    import concourse.bacc as bacc
    import concourse.bass as bass
    import concourse.mybir as mybir
    from concourse import tile

    f32 = mybir.dt.float32
    f16 = mybir.dt.float16
    f8 = mybir.dt.float8e4
    i16 = mybir.dt.int16
    AF = mybir.ActivationFunctionType
    OP = mybir.AluOpType

    W, W0, P, K = cfg["W"], cfg["W0"], cfg["P"], cfg["K"]
    k_lo, k_hi, gw = cfg["k_lo"], cfg["k_hi"], cfg["gw"]
    G = W // gw
    T = gw * K                  # tiles per group
    GCAP = 1024                 # max gather descriptors per SWDGE call (HW limit)
    NLO = gw * k_lo * 128       # lo gather idxs per group
    NHI = gw * k_hi * 128
    W1c = W - W0                # windows in chunk 1
    CH0 = NCORES * W0 * 128     # rows in chunk-0 table
    CH1 = NCORES * W1c * 128

    nc = bacc.Bacc("TRN2", target_bir_lowering=False, debug=False, num_devices=NCORES,
                   dynamic_dma_scratch_size=32768)

    # ---- external inputs ----
    xT = nc.dram_tensor("xT", [IN_CH, P], f32, kind="ExternalInput")
    w1e = nc.dram_tensor("w1e", [IN_CH, 136], f32, kind="ExternalInput")
    w2e = nc.dram_tensor("w2e", [HC1, 66], f32, kind="ExternalInput")
    idx_d = nc.dram_tensor("idx", [128, W * K * 8], i16, kind="ExternalInput")
    es_d = nc.dram_tensor("es", [128, W * K * 256], f8, kind="ExternalInput")
    ident_d = nc.dram_tensor("ident", [128, 128], f16, kind="ExternalInput")
    b1_d = nc.dram_tensor("b1", [128, HC1], f32, kind="ExternalInput")
    b2_d = nc.dram_tensor("b2", [128, C2], f32, kind="ExternalInput")
    out_d = nc.dram_tensor("out", [P, C2], f32, kind="ExternalOutput")

    with tile.TileContext(nc) as tc:
        with (
            tc.tile_pool(name="const", bufs=1) as cpool,
            tc.tile_pool(name="xc", bufs=3) as xcpool,
            tc.tile_pool(name="rows", bufs=3) as rowpool,
            tc.tile_pool(name="gather", bufs=2) as gpool,
            tc.tile_pool(name="onehot", bufs=2) as opool,
            tc.tile_pool(name="rmat", bufs=2) as rpool,
            tc.tile_pool(name="scal", bufs=3) as spool,
            tc.tile_pool(name="idx", bufs=3) as ipool,
            tc.tile_pool(name="epi", bufs=3) as epool,
            tc.tile_pool(name="psA", bufs=2, space="PSUM") as psA,
            tc.tile_pool(name="psT", bufs=1, space="PSUM") as psTp,
            tc.tile_pool(name="ps2", bufs=1, space="PSUM") as ps2p,
            tc.tile_pool(name="psW", bufs=3, space="PSUM") as psW,
            tc.tile_pool(name="psad", bufs=1, space="PSUM") as psad,
            tc.tile_pool(name="dram", bufs=1, space="DRAM") as dpool,
        ):
            # ---- constants to SBUF ----
            w1e_sb = cpool.tile([IN_CH, 136], f32, tag="w1e")
            nc.sync.dma_start(w1e_sb[:], w1e[:])
            w2e_sb = cpool.tile([HC1, 66], f32, tag="w2e")
            nc.sync.dma_start(w2e_sb[:], w2e[:])
            w2e_f16 = cpool.tile([HC1, 66], f16, tag="w2e16")
            nc.vector.tensor_copy(w2e_f16[:], w2e_sb[:])
            ident = cpool.tile([128, 128], f16, tag="ident")
            nc.sync.dma_start(ident[:], ident_d[:])
            b1_sb = cpool.tile([128, HC1], f32, tag="b1")
            nc.sync.dma_start(b1_sb[:], b1_d[:])
            b2_sb = cpool.tile([128, C2], f32, tag="b2")
            nc.sync.dma_start(b2_sb[:], b2_d[:])
            adsb1 = cpool.tile([128, W, H1], f16, tag="adsb1")
            adsb2 = cpool.tile([128, W, H2], f16, tag="adsb2")
            x2T = cpool.tile([128, P], f16, tag="x2T")

            # ---- DRAM tables: 2 chunks, each AllGathered by ONE collective ----
            t1s = [dpool.tile([W0 * 128, ROW1], f16, tag="t1s0", name="t1s0"),
                   dpool.tile([W1c * 128, ROW1], f16, tag="t1s1", name="t1s1")]
            t1f = [dpool.tile([CH0, ROW1], f16, tag="t1f0", name="t1f0", addr_space="Shared"),
                   dpool.tile([CH1, ROW1], f16, tag="t1f1", name="t1f1", addr_space="Shared")]
            t2s = [dpool.tile([W0 * 128, ROW2], f16, tag="t2s0", name="t2s0"),
                   dpool.tile([W1c * 128, ROW2], f16, tag="t2s1", name="t2s1")]
            t2f = [dpool.tile([CH0, ROW2], f16, tag="t2f0", name="t2f0", addr_space="Shared"),
                   dpool.tile([CH1, ROW2], f16, tag="t2f1", name="t2f1", addr_space="Shared")]

            def ag_chunk(slice_t, full_t):
                nc.gpsimd.collective_compute(
                    "AllGather", mybir.AluOpType.bypass,
                    replica_groups=[list(range(NCORES))],
                    ins=[slice_t.opt()], outs=[full_t.opt()],
                )

            def wslot(w):
                # (chunk index, row offset of window w in its chunk slice)
                return (0, w * 128) if w < W0 else (1, (w - W0) * 128)

            # ================= phase A: layer-1 node matmul =================
            assert W % 2 == 0 and W0 % 2 == 0
            for wp in range(W // 2):
                xc = xcpool.tile([IN_CH, 256], f32, tag="xc")
                nc.sync.dma_start(xc[:], xT[:, wp * 256:(wp + 1) * 256])
                rows = rowpool.tile([128, 2, 136], f16, tag="rows1")
                for half in range(2):
                    w = wp * 2 + half
                    ps = psA.tile([128, 136], f32, tag="ps_node")
                    nc.tensor.matmul(ps[:], lhsT=xc[:, half * 128:(half + 1) * 128],
                                     rhs=w1e_sb[:], start=True, stop=True)
                    nc.scalar.activation(rows[:, half, :], ps[:], AF.Copy)
                    nc.vector.tensor_copy(adsb1[:, w, :], ps[:, 132:136])
                w = wp * 2
                ci, ro = wslot(w)
                nc.sync.dma_start(
                    t1s[ci][ro:ro + 256, 0:136]
                    .rearrange("(w p) c -> p w c", w=2), rows[:])
                if w + 1 == W0 - 1:
                    ag_chunk(t1s[0], t1f[0])
                elif w + 1 == W - 1:
                    ag_chunk(t1s[1], t1f[1])

            # ================= edge phase (shared for both layers) ==========
            def edge_phase(layer):
                if layer == 1:
                    tabs, row, heads, ch = t1f, ROW1, H1, C1
                    adsb = adsb1
                    a_s_off = 128
                    rcols = HC1 + H1  # 132
                else:
                    tabs, row, heads, ch = t2f, ROW2, H2, C2
                    adsb = adsb2
                    a_s_off = 64
                    rcols = C2 + H2  # 65
                hc = heads * ch
                nh = heads  # scalar cols per tile
                NIX = (NLO + NHI) // 16
                for g in range(G):
                    w0 = g * gw
                    # --- gather (merged idx load: [lo | hi] per group) ---
                    Gt = gpool.tile([128, T, row], f16, tag=f"G{layer}")
                    ix = ipool.tile([128, NIX], i16, tag="ix")
                    nc.sync.dma_start(ix[:], idx_d[:, g * NIX:(g + 1) * NIX])
                    for off in range(0, NLO, GCAP):
                        sz = min(GCAP, NLO - off)
                        nc.gpsimd.dma_gather(
                            out_ap=Gt[:, off // 128:(off + sz) // 128, :],
                            in_ap=tabs[0][:],
                            idxs_ap=ix[:, off // 16:(off + sz) // 16],
                            num_idxs=sz, num_idxs_reg=sz,
                            elem_size=row)
                    for off in range(0, NHI, GCAP):
                        sz = min(GCAP, NHI - off)
                        nc.gpsimd.dma_gather(
                            out_ap=Gt[:, gw * k_lo + off // 128:gw * k_lo + (off + sz) // 128, :],
                            in_ap=tabs[1][:],
                            idxs_ap=ix[:, (NLO + off) // 16:(NLO + off + sz) // 16],
                            num_idxs=sz, num_idxs_reg=sz,
                            elem_size=row)
                    # --- one-hot stream (fp8, host-precomputed, [e1 | sm]) ---
                    es = opool.tile([128, 2 * T, 128], f8, tag="es")
                    nc.sync.dma_start(
                        es[:].rearrange("p t j -> p (t j)"),
                        es_d[:, g * 2 * T * 128:(g + 1) * 2 * T * 128])

                    # --- a_d broadcast matmuls ---
                    pad = psad.tile([128, T * nh], f32, tag="pad")
                    for t in range(T):
                        w = w0 + (t // k_lo if t < gw * k_lo else (t - gw * k_lo) // k_hi)
                        nc.tensor.matmul(pad[:, t * nh:(t + 1) * nh],
                                         lhsT=es[:, T + t, :], rhs=adsb[:, w, :],
                                         start=True, stop=True)
                    # --- per-edge scalars: z = a_s + a_d; p = exp(lrelu(z)) ---
                    z = spool.tile([128, T, nh], f32, tag="z")
                    nc.vector.tensor_add(z[:], Gt[:, :, a_s_off:a_s_off + nh],
                                         pad[:].rearrange("p (t h) -> p t h", h=nh))
                    zl = spool.tile([128, T, nh], f32, tag="zl")
                    nc.scalar.activation(zl[:], z[:], AF.Prelu, alpha=NEG_SLOPE)
                    # psc expanded across channels (ACT broadcast-read), f16
                    pexp = spool.tile([128, T, hc], f16, tag="pexp")
                    nc.scalar.activation(
                        pexp[:].rearrange("p t (h c) -> p t h c", h=nh),
                        zl[:].rearrange("p t (h one) -> p t h one", one=1)
                             .broadcast_to([128, T, nh, ch]),
                        AF.Exp)
                    # --- R = [p*h | p] ---
                    R = rpool.tile([128, T, rcols], f16, tag="R")
                    nc.vector.tensor_mul(R[:, :, 0:hc], Gt[:, :, 0:hc], pexp[:])
                    nc.scalar.activation(R[:, :, hc:hc + nh], zl[:], AF.Exp)
                    # --- segment-sum matmuls ---
                    pw = [psW.tile([128, rcols], f32, tag="psW", name=f"pw{layer}_{g}_{wi}")[:]
                          for wi in range(gw)]
                    for t in range(T):
                        if t < gw * k_lo:
                            wi, first = divmod(t, k_lo)
                            is_first = first == 0
                            is_last = (first == k_lo - 1) and k_hi == 0
                        else:
                            wi, r = divmod(t - gw * k_lo, k_hi)
                            is_first = False
                            is_last = r == k_hi - 1
                        nc.tensor.matmul(pw[wi], lhsT=es[:, t, :], rhs=R[:, t, :],
                                         start=is_first, stop=is_last)
                    # --- epilogue per window (DMAs batched per group) ---
                    if layer == 1:
                        rows2 = rowpool.tile([128, gw, 66], f16, tag="rows2")
                    else:
                        og = epool.tile([128, gw, hc], f32, tag="og")
                    for wi in range(gw):
                        w = w0 + wi
                        den = epool.tile([128, nh], f32, tag="den")
                        nc.vector.tensor_scalar_add(den[:], pw[wi][:, hc:hc + nh], EPS)
                        rec = epool.tile([128, nh], f32, tag="rec")
                        nc.vector.reciprocal(rec[:], den[:])
                        if layer == 1:
                            o = epool.tile([128, hc], f32, tag="o")
                            nc.vector.tensor_mul(
                                o[:].rearrange("p (h c) -> p h c", h=heads),
                                pw[wi][:, 0:hc].rearrange("p (h c) -> p h c", h=heads),
                                rec[:].broadcast_to([128, heads, ch]))
                            nc.vector.tensor_add(o[:], o[:], b1_sb[:])
                            # ELU: relu(o) + exp(min(o,0)) - 1
                            r_ = epool.tile([128, hc], f32, tag="relu")
                            nc.scalar.activation(r_[:], o[:], AF.Relu)
                            m_ = epool.tile([128, hc], f32, tag="mneg")
                            nc.vector.tensor_scalar_min(m_[:], o[:], 0.0)
                            nc.scalar.activation(m_[:], m_[:], AF.Exp)
                            act = epool.tile([128, hc], f16, tag="act")
                            nc.vector.scalar_tensor_tensor(
                                act[:], r_[:], -1.0, m_[:], OP.add, OP.add)
                            # transpose into x2T; layer-2 node matmul + table row
                            psT = psTp.tile([128, 128], f16, tag="psT")
                            nc.tensor.transpose(psT[:], act[:], ident[:])
                            nc.vector.tensor_copy(x2T[:, w * 128:(w + 1) * 128], psT[:])
                            ps2 = ps2p.tile([128, 66], f32, tag="ps2")
                            nc.tensor.matmul(ps2[:], lhsT=x2T[:, w * 128:(w + 1) * 128],
                                             rhs=w2e_f16[:], start=True, stop=True)
                            nc.scalar.activation(rows2[:, wi, :], ps2[:], AF.Copy)
                            nc.vector.tensor_copy(adsb2[:, w, :], ps2[:, 65:66])
                        else:
                            nc.vector.scalar_tensor_tensor(
                                og[:, wi, :], pw[wi][:, 0:hc], rec[:], b2_sb[:],
                                OP.mult, OP.add)
                    if layer == 1:
                        ci, ro = wslot(w0)
                        nc.sync.dma_start(
                            t2s[ci][ro:ro + gw * 128, 0:66]
                            .rearrange("(w p) c -> p w c", w=gw), rows2[:])
                        if w0 + gw == W0:
                            ag_chunk(t2s[0], t2f[0])
                        elif w0 + gw == W:
                            ag_chunk(t2s[1], t2f[1])
                    else:
                        nc.sync.dma_start(
                            out_d[w0 * 128:(w0 + gw) * 128, :]
                            .rearrange("(w p) c -> p w c", w=gw), og[:])

            edge_phase(1)
            edge_phase(2)

    nc.compile()
    return nc


# ---------------------------------------------------------------------------
# Entry point
# ---------------------------------------------------------------------------

_CACHE = {}


def _prepare(inputs):
    x = np.ascontiguousarray(np.asarray(inputs["x"], np.float32))
    ei = np.asarray(inputs["edge_index"])
    n_nodes = x.shape[0]
    return pick_config(x, ei, n_nodes)


def _weights_ext(inputs):
    W1 = np.asarray(inputs["W1"], np.float32)
    as1 = np.asarray(inputs["att_src1"], np.float32)
    ad1 = np.asarray(inputs["att_dst1"], np.float32)
    W2 = np.asarray(inputs["W2"], np.float32)
    as2 = np.asarray(inputs["att_src2"], np.float32)
    ad2 = np.asarray(inputs["att_dst2"], np.float32)
    As = np.zeros((HC1, H1), np.float32)
    Ad = np.zeros((HC1, H1), np.float32)
    for h in range(H1):
        As[h * C1:(h + 1) * C1, h] = as1[0, h]
        Ad[h * C1:(h + 1) * C1, h] = ad1[0, h]
    w1e = np.concatenate([W1, W1 @ As, W1 @ Ad], axis=1)           # [128,136]
    w2e = np.concatenate([W2, W2 @ as2[0].T, W2 @ ad2[0].T], axis=1)  # [128,66]
    return np.ascontiguousarray(w1e), np.ascontiguousarray(w2e)


def kernel(**inputs):
    from concourse.bass_utils import run_bass_kernel_spmd

    prep = _prepare(inputs)
    key = (prep["W"], prep["W0"], prep["k_lo"], prep["k_hi"], prep["gw"])
    if key not in _CACHE:
        _CACHE[key] = build_program(dict(
            W=prep["W"], W0=prep["W0"], P=prep["P"], K=prep["K"],
            k_lo=prep["k_lo"], k_hi=prep["k_hi"], gw=prep["gw"]))
    nc = _CACHE[key]

    in_maps = build_in_maps(inputs, prep)
    res = run_bass_kernel_spmd(nc, in_maps, core_ids=list(range(NCORES)))
    return assemble_output(res.results, prep)


def build_in_maps(inputs, prep):
    import concourse.mybir as mybir
    f8np = mybir.dt.np(mybir.dt.float8e4)
    x = np.ascontiguousarray(np.asarray(inputs["x"], np.float32))
    b1 = np.tile(np.asarray(inputs["b1"], np.float32).reshape(1, HC1), (128, 1))
    b2 = np.tile(np.asarray(inputs["b2"], np.float32).reshape(1, C2), (128, 1))
    w1e, w2e = _weights_ext(inputs)
    n_nodes, P, W = prep["n_nodes"], prep["P"], prep["W"]
    po = prep["po"]
    ident = np.eye(128, dtype=np.float16)
    in_maps = []
    # xT_pad per core: columns = core-local padded slots (window-major)
    xT_all = np.zeros((NCORES, IN_CH, P), np.float32)
    node_ids = np.arange(n_nodes)
    c_of = po // P
    col = po % P
    xT_all[c_of, :, col] = x[node_ids]  # fancy: sets [ch] vectors
    for c in range(NCORES):
        m = prep["meta"][c]
        im = dict(
            xT=np.ascontiguousarray(xT_all[c]),
            w1e=w1e, w2e=w2e,
            idx=np.ascontiguousarray(m["idx"]),
            es=m["es"].view(f8np),
            ident=ident,
            b1=b1, b2=b2,
        )
        in_maps.append(im)
    return in_maps


def assemble_output(results, prep):
    full = np.concatenate([results[c]["out"] for c in range(NCORES)], axis=0)
    return np.ascontiguousarray(full[prep["po"]]).astype(np.float32)


# revision 25
# speedup vs baseline: 2.4072x; 2.4072x over previous
"""Trainium2 Bass kernel for a 2-layer GAT encoder (edge-softmax message passing).

Strategy (8 NeuronCores, SPMD single program):
- dst-node partition across cores; host packs each core's dst nodes into
  fixed-count "windows" (<=128 nodes each) and edges into fixed-count
  128-edge tiles per window (K_LO tiles for src in the low half of the
  padded node space, K_HI for the high half -- dma_gather indices are int16).
- Node phase: h_ext = x @ [W | W@att_src | W@att_dst] per core slice,
  fp16 row table written to DRAM in chunks, chunked AllGather pipelined
  with the node matmuls -> full table on every core.
- Edge phase per 128-edge tile: dma_gather rows by src; one-hot matrices
  (fp8, host-precomputed, streamed from DRAM); per-edge a_d via one-hot
  matmul; p = exp(leaky_relu(a_s+a_d)) on the Scalar engine (softmax shift
  invariance makes the segment-max subtraction unnecessary); segment-sum of
  [p*h | p] via one-hot matmul accumulated in PSUM per window; epilogue
  divides and applies bias/ELU. Layer-2 node matmul + table write is
  interleaved per-window into the layer-1 edge phase, with the second
  chunked AllGather riding on top.
- Output rows are window-padded; host de-permutes to the original node order.
"""

import numpy as np

NCORES = 8
HALF = 32768          # int16 gather index limit
ROW1 = 256            # fp16 elems per layer-1 table row (512B): h128|a_s4|a_d4|pad
ROW2 = 128            # fp16 elems per layer-2 table row (256B): h64|a_s|pad
H1, C1 = 4, 32
H2, C2 = 1, 64
IN_CH = 128
HC1 = H1 * C1         # 128
NEG_SLOPE = 0.2
EPS = 1e-16


# ---------------------------------------------------------------------------
# Host-side preprocessing
# ---------------------------------------------------------------------------

def _pack_windows(src, dst, n_nodes, k_lo, k_hi, boundary_arr, per_core,
                  _cache={}):
    """Greedy-pack each core's dst nodes into windows (<=128 nodes, <=k_lo
    lo-tiles, <=k_hi hi-tiles). lo/hi = src node below/above its own core's
    chunk boundary (boundary_arr[core]). Edges must be sorted by dst."""
    key = id(src)
    if _cache.get("key") != key:
        order = np.argsort(dst, kind="stable")
        src_s, dst_s = src[order], dst[order]
        counts = np.bincount(dst_s, minlength=n_nodes)
        starts = np.concatenate([[0], np.cumsum(counts)])
        _cache.update(key=key, src_s=src_s, dst_s=dst_s, starts=starts)
    src_s, dst_s, starts = _cache["src_s"], _cache["dst_s"], _cache["starts"]
    lo_m_s = src_s < boundary_arr[src_s // per_core]
    nlo_n = np.bincount(dst_s[lo_m_s], minlength=n_nodes)
    nall_n = np.bincount(dst_s, minlength=n_nodes)
    nhi_n = nall_n - nlo_n
    cap_lo, cap_hi = k_lo * 128, k_hi * 128
    cores = []
    for c in range(NCORES):
        lo_n, hi_n = c * per_core, (c + 1) * per_core
        bounds = []  # window node ranges [a, b)
        a = lo_n
        cl = ch = cn = 0
        for n in range(lo_n, hi_n):
            el, eh = int(nlo_n[n]), int(nhi_n[n])
            if cn >= 128 or cl + el > cap_lo or ch + eh > cap_hi:
                assert n > a, "single node exceeds tile budget"
                bounds.append((a, n))
                a, cl, ch, cn = n, 0, 0, 0
            cl += el; ch += eh; cn += 1
        bounds.append((a, hi_n))
        wins = []
        for (a, b) in bounds:
            e0, e1_ = starts[a], starts[b]
            es = src_s[e0:e1_]
            dr = (dst_s[e0:e1_] - a).astype(np.int32)
            lm = lo_m_s[e0:e1_]
            wins.append((a, b, es[lm], es[~lm], dr[lm], dr[~lm]))
        cores.append(wins)
    return cores


def _win_edges(src, dst, a, b, n_nodes, per_core):
    c = _pack_windows.__defaults__[0]  # cache dict
    starts, src_s = c["starts"], c["src_s"]
    return src_s[starts[a]:starts[b]]


def _win_tuple(src, dst, a, b, n_nodes, per_core, pi, CH0):
    c = _pack_windows.__defaults__[0]
    starts, src_s, dst_s = c["starts"], c["src_s"], c["dst_s"]
    e0, e1_ = starts[a], starts[b]
    es = src_s[e0:e1_]
    dr = (dst_s[e0:e1_] - a).astype(np.int32)
    lm = pi[es] < CH0
    return (a, b, es[lm], es[~lm], dr[lm], dr[~lm])


def _layout(cores, gw, n_nodes, per_core):
    """Window count, chunk split, and the two permutations.
    pi: node -> global chunk-major table row (chunk 0 = windows [0,W0) of
    every core, chunk 1 = the rest).  po: node -> core-local padded slot
    (c*P + w*128 + s), the output row order."""
    W = max(len(w) for w in cores)
    W = ((W + gw - 1) // gw) * gw
    W0 = ((W // 2 + 1) // 2) * 2  # even: chunk boundary on a group boundary
    P = W * 128
    CH0 = NCORES * W0 * 128
    pi = np.zeros(n_nodes, np.int64)
    po = np.zeros(n_nodes, np.int64)
    boundary_arr = np.full(NCORES, 0, np.int64)
    for c, wins in enumerate(cores):
        boundary_arr[c] = (c + 1) * per_core
        for w, (a, b, *_r) in enumerate(wins):
            ids = np.arange(a, b)
            po[ids] = c * P + w * 128 + (ids - a)
            if w < W0:
                pi[ids] = c * W0 * 128 + w * 128 + (ids - a)
            else:
                pi[ids] = CH0 + c * (W - W0) * 128 + (w - W0) * 128 + (ids - a)
            if w == W0:
                boundary_arr[c] = a
    return W, W0, P, CH0, pi, po, boundary_arr


def host_prep(x, edge_index, n_nodes, k_lo, k_hi, gw):
    """Build the permutations, per-core metadata and index arrays."""
    src = np.ascontiguousarray(edge_index[0]).astype(np.int64)
    dst = np.ascontiguousarray(edge_index[1]).astype(np.int64)
    per_core = n_nodes // NCORES
    assert per_core * NCORES == n_nodes

    # fixpoint on the per-core lo/hi chunk boundaries: the packer's mask only
    # shapes the tile budgets; final buckets use the true pi and re-verify.
    boundary_arr = np.array([c * per_core + per_core // 2 for c in range(NCORES)],
                            np.int64)
    ok = False
    for it in range(14):
        cores = _pack_windows(src, dst, n_nodes, k_lo, k_hi, boundary_arr, per_core)
        W, W0, P, CH0, pi, po, nb = _layout(cores, gw, n_nodes, per_core)
        CH1 = NCORES * (W - W0) * 128
        if CH0 < 32768 and CH1 < 32768:
            # repair: shift tail nodes of overflowing windows (TRUE buckets)
            # into the next window, then re-layout; a few rounds suffice
            for _rep in range(4):
                W, W0, P, CH0, pi, po, _nb2 = _layout(cores, gw, n_nodes, per_core)
                CH1 = NCORES * (W - W0) * 128
                if CH0 >= 32768 or CH1 >= 32768:
                    break
                moved = 0
                bad = False
                for c in range(NCORES):
                    wins = cores[c]
                    nw = []
                    carry = 0  # nodes pushed into the current window from the left
                    for wi_, (a, b, lo_s, hi_s, lo_dr, hi_dr) in enumerate(wins):
                        a -= carry
                        carry = 0
                        while True:
                            es_ = _win_edges(src, dst, a, b, n_nodes, per_core)
                            nlo = int((pi[es_] < CH0).sum())
                            nhi = es_.shape[0] - nlo
                            if (b - a) <= 128 and nlo <= k_lo * 128 and nhi <= k_hi * 128:
                                break
                            if wi_ == len(wins) - 1 or b - a <= 1:
                                bad = True
                                break
                            b -= 1
                            carry += 1
                            moved += 1
                        if bad:
                            break
                        nw.append((a, b))
                    if bad:
                        break
                    cores[c] = [_win_tuple(src, dst, a, b, n_nodes, per_core, pi, CH0)
                                for (a, b) in nw]
                if bad:
                    break
                if moved == 0:
                    ok = True
                    break
            if ok:
                break
        boundary_arr = nb
    assert ok, "chunk-boundary fixpoint failed"
    assert P * NCORES <= 65536

    K = k_lo + k_hi
    meta = []
    G = W // gw
    for c, wins in enumerate(cores):
        idx_lo = np.zeros((W, k_lo * 128), np.int16)
        idx_hi = np.zeros((W, k_hi * 128), np.int16)
        drel = np.full((W, K, 128), 255, np.int32)  # [window, tile-in-window, slot]
        for w, (a, b, lo_s, hi_s, lo_dr, hi_dr) in enumerate(wins):
            es = np.concatenate([lo_s, hi_s])
            dr = np.concatenate([lo_dr, hi_dr])
            lm = pi[es] < CH0                      # true buckets
            pl = pi[es[lm]]
            ph = pi[es[~lm]] - CH0
            ld, hd = dr[lm], dr[~lm]
            assert (pl < CH0).all() and (ph >= 0).all() and (ph < CH1).all()
            idx_lo[w, :len(pl)] = pl.astype(np.int16)
            idx_hi[w, :len(ph)] = ph.astype(np.int16)
            dr_pad = np.full(k_lo * 128, 255, np.int32)
            dr_pad[:len(ld)] = ld
            drel[w, :k_lo] = dr_pad.reshape(k_lo, 128)
            dr_pad = np.full(k_hi * 128, 255, np.int32)
            dr_pad[:len(hd)] = hd
            drel[w, k_lo:] = dr_pad.reshape(k_hi, 128)
        # group-tile order: per group: [lo tiles of gw windows][hi tiles of gw windows]
        tile_order = []  # (window, tile-in-window-index)
        for g in range(G):
            for w in range(g * gw, (g + 1) * gw):
                tile_order += [(w, t) for t in range(k_lo)]
            for w in range(g * gw, (g + 1) * gw):
                tile_order += [(w, k_lo + t) for t in range(k_hi)]
        to = np.array(tile_order)
        drel_t = drel[to[:, 0], to[:, 1]]            # [W*K, 128]
        # one-hot matrices, fp8(e4m3) encoded as raw uint8 bit patterns:
        # 1.0 -> 0x38, 0.0 -> 0x00.
        j = np.arange(128, dtype=np.int32)
        # e1[p, tile, j] = (drel_t[tile, p] == j): edge-slot on partitions
        e1 = (drel_t.T[:, :, None] == j[None, None, :]).astype(np.uint8) * 0x38
        # sm[p, tile, e] = (p == drel_t[tile, e]): dst-rel on partitions
        sm = (j[:, None, None] == drel_t[None, :, :]).astype(np.uint8) * 0x38
        # merged one-hot stream: per group [e1 tiles | sm tiles], fp8 bytes
        T_ = gw * K
        e1g = e1.reshape(128, G, T_, 128)
        smg = sm.reshape(128, G, T_, 128)
        es_ = np.concatenate([e1g, smg], axis=2)         # [128, G, 2T, 128]
        # merged idx stream: per group [lo idxs | hi idxs], wrapped by 16
        def wrap16(a):
            # idx j lives at [j%16, j//16], replicated into all 8 Q7 core
            # partition groups (HW reads each group independently)
            return np.ascontiguousarray(np.tile(a.reshape(-1, 16).T, (8, 1)))
        nlo, nhi = gw * k_lo * 128, gw * k_hi * 128
        ilg = idx_lo.reshape(G, nlo)
        ihg = idx_hi.reshape(G, nhi)
        idx = np.concatenate([ilg, ihg], axis=1)          # [G, nlo+nhi]
        meta.append(dict(
            idx=wrap16(idx),
            e1=np.ascontiguousarray(e1.reshape(128, -1)),
            sm=np.ascontiguousarray(sm.reshape(128, -1)),
            es=np.ascontiguousarray(es_.reshape(128, -1)),
        ))
    return dict(cores=cores, pi=pi, po=po, W=W, W0=W0, P=P, K=K, k_lo=k_lo,
                k_hi=k_hi, gw=gw, meta=meta, n_nodes=n_nodes, per_core=per_core)


def pick_config(x, edge_index, n_nodes):
    """Try candidate (k_lo, k_hi) packings in predicted-cost order; first
    feasible wins. Cost ~ gather bytes + SWDGE call fixed overhead + per-
    window epilogue overhead."""
    gw = 2
    E = edge_index.shape[1]
    lam = E / n_nodes * 128
    base = max(int(np.ceil(lam * 0.5 / 128)), 1)
    cands = []
    for dlo in (-1, 0, 1, 2):
        for dhi in (-1, 0, 1, 2):
            kl, kh = base + dlo, base + dhi
            if kl >= 1 and kh >= 1:
                K_ = kl + kh
                West = int(np.ceil(E / NCORES / 128 / K_ * 1.07 / gw)) * gw
                calls = (West // gw) * (-(-gw * kl * 128 // 1024) + -(-gw * kh * 128 // 1024))
                cost = West * K_ * 50 + calls * 994 + West * 1200
                cands.append((cost, kl, kh))
    cands.sort()
    for _cost, k_lo, k_hi in cands:
        try:
            return host_prep(x, edge_index, n_nodes, k_lo, k_hi, gw)
        except AssertionError:
            continue
    raise AssertionError("no feasible packing found")


# ---------------------------------------------------------------------------
# Bass program
# ---------------------------------------------------------------------------

def build_program(cfg):
    import concourse.bacc as bacc
    import concourse.bass as bass
    import concourse.mybir as mybir
    from concourse import tile

    f32 = mybir.dt.float32
    f16 = mybir.dt.float16
    f8 = mybir.dt.float8e4
    i16 = mybir.dt.int16
    AF = mybir.ActivationFunctionType
    OP = mybir.AluOpType

    W, W0, P, K = cfg["W"], cfg["W0"], cfg["P"], cfg["K"]
    k_lo, k_hi, gw = cfg["k_lo"], cfg["k_hi"], cfg["gw"]
    G = W // gw
    T = gw * K                  # tiles per group
    GCAP = 1024                 # max gather descriptors per SWDGE call (HW limit)
    NLO = gw * k_lo * 128       # lo gather idxs per group
    NHI = gw * k_hi * 128
    W1c = W - W0                # windows in chunk 1
    CH0 = NCORES * W0 * 128     # rows in chunk-0 table
    CH1 = NCORES * W1c * 128

    nc = bacc.Bacc("TRN2", target_bir_lowering=False, debug=False, num_devices=NCORES,
                   dynamic_dma_scratch_size=32768)

    # ---- external inputs ----
    xT = nc.dram_tensor("xT", [IN_CH, P], f32, kind="ExternalInput")
    w1e = nc.dram_tensor("w1e", [IN_CH, 136], f32, kind="ExternalInput")
    w2e = nc.dram_tensor("w2e", [HC1, 66], f32, kind="ExternalInput")
    idx_d = nc.dram_tensor("idx", [128, W * K * 8], i16, kind="ExternalInput")
    es_d = nc.dram_tensor("es", [128, W * K * 256], f8, kind="ExternalInput")
    ident_d = nc.dram_tensor("ident", [128, 128], f16, kind="ExternalInput")
    b1_d = nc.dram_tensor("b1", [128, HC1], f32, kind="ExternalInput")
    b2_d = nc.dram_tensor("b2", [128, C2], f32, kind="ExternalInput")
    out_d = nc.dram_tensor("out", [P, C2], f32, kind="ExternalOutput")

    with tile.TileContext(nc) as tc:
        with (
            tc.tile_pool(name="const", bufs=1) as cpool,
            tc.tile_pool(name="xc", bufs=3) as xcpool,
            tc.tile_pool(name="rows", bufs=3) as rowpool,
            tc.tile_pool(name="gather", bufs=2) as gpool,
            tc.tile_pool(name="onehot", bufs=2) as opool,
            tc.tile_pool(name="rmat", bufs=2) as rpool,
            tc.tile_pool(name="scal", bufs=3) as spool,
            tc.tile_pool(name="idx", bufs=3) as ipool,
            tc.tile_pool(name="epi", bufs=3) as epool,
            tc.tile_pool(name="psA", bufs=2, space="PSUM") as psA,
            tc.tile_pool(name="psT", bufs=1, space="PSUM") as psTp,
            tc.tile_pool(name="ps2", bufs=1, space="PSUM") as ps2p,
            tc.tile_pool(name="psW", bufs=3, space="PSUM") as psW,
            tc.tile_pool(name="psad", bufs=1, space="PSUM") as psad,
            tc.tile_pool(name="dram", bufs=1, space="DRAM") as dpool,
        ):
            # ---- constants to SBUF ----
            w1e_sb = cpool.tile([IN_CH, 136], f32, tag="w1e")
            nc.sync.dma_start(w1e_sb[:], w1e[:])
            w2e_sb = cpool.tile([HC1, 66], f32, tag="w2e")
            nc.sync.dma_start(w2e_sb[:], w2e[:])
            w2e_f16 = cpool.tile([HC1, 66], f16, tag="w2e16")
            nc.vector.tensor_copy(w2e_f16[:], w2e_sb[:])
            ident = cpool.tile([128, 128], f16, tag="ident")
            nc.sync.dma_start(ident[:], ident_d[:])
            b1_sb = cpool.tile([128, HC1], f32, tag="b1")
            nc.sync.dma_start(b1_sb[:], b1_d[:])
            b2_sb = cpool.tile([128, C2], f32, tag="b2")
            nc.sync.dma_start(b2_sb[:], b2_d[:])
            adsb1 = cpool.tile([128, W, H1], f16, tag="adsb1")
            adsb2 = cpool.tile([128, W, H2], f16, tag="adsb2")
            x2T = cpool.tile([128, P], f16, tag="x2T")

            # ---- DRAM tables: 2 chunks, each AllGathered by ONE collective ----
            t1s = [dpool.tile([W0 * 128, ROW1], f16, tag="t1s0", name="t1s0"),
                   dpool.tile([W1c * 128, ROW1], f16, tag="t1s1", name="t1s1")]
            t1f = [dpool.tile([CH0, ROW1], f16, tag="t1f0", name="t1f0", addr_space="Shared"),
                   dpool.tile([CH1, ROW1], f16, tag="t1f1", name="t1f1", addr_space="Shared")]
            t2s = [dpool.tile([W0 * 128, ROW2], f16, tag="t2s0", name="t2s0"),
                   dpool.tile([W1c * 128, ROW2], f16, tag="t2s1", name="t2s1")]
            t2f = [dpool.tile([CH0, ROW2], f16, tag="t2f0", name="t2f0", addr_space="Shared"),
                   dpool.tile([CH1, ROW2], f16, tag="t2f1", name="t2f1", addr_space="Shared")]

            def ag_chunk(slice_t, full_t):
                nc.gpsimd.collective_compute(
                    "AllGather", mybir.AluOpType.bypass,
                    replica_groups=[list(range(NCORES))],
                    ins=[slice_t.opt()], outs=[full_t.opt()],
                )

            def wslot(w):
                # (chunk index, row offset of window w in its chunk slice)
                return (0, w * 128) if w < W0 else (1, (w - W0) * 128)

            # ================= phase A: layer-1 node matmul =================
            assert W % 2 == 0 and W0 % 2 == 0
            for wp in range(W // 2):
                xc = xcpool.tile([IN_CH, 256], f32, tag="xc")
                nc.sync.dma_start(xc[:], xT[:, wp * 256:(wp + 1) * 256])
                rows = rowpool.tile([128, 2, 136], f16, tag="rows1")
                for half in range(2):
                    w = wp * 2 + half
                    ps = psA.tile([128, 136], f32, tag="ps_node")
                    nc.tensor.matmul(ps[:], lhsT=xc[:, half * 128:(half + 1) * 128],
                                     rhs=w1e_sb[:], start=True, stop=True)
                    nc.scalar.activation(rows[:, half, :], ps[:], AF.Copy)
                    nc.vector.tensor_copy(adsb1[:, w, :], ps[:, 132:136])
                w = wp * 2
                ci, ro = wslot(w)
                nc.sync.dma_start(
                    t1s[ci][ro:ro + 256, 0:136]
                    .rearrange("(w p) c -> p w c", w=2), rows[:])
                if w + 1 == W0 - 1:
                    ag_chunk(t1s[0], t1f[0])
                elif w + 1 == W - 1:
                    ag_chunk(t1s[1], t1f[1])

            # ================= edge phase (shared for both layers) ==========
            def edge_phase(layer):
                if layer == 1:
                    tabs, row, heads, ch = t1f, ROW1, H1, C1
                    adsb = adsb1
                    a_s_off = 128
                    rcols = HC1 + H1  # 132
                else:
                    tabs, row, heads, ch = t2f, ROW2, H2, C2
                    adsb = adsb2
                    a_s_off = 64
                    rcols = C2 + H2  # 65
                hc = heads * ch
                nh = heads  # scalar cols per tile
                NIX = (NLO + NHI) // 16
                for g in range(G):
                    w0 = g * gw
                    # --- gather (merged idx load: [lo | hi] per group) ---
                    Gt = gpool.tile([128, T, row], f16, tag=f"G{layer}")
                    ix = ipool.tile([128, NIX], i16, tag="ix")
                    nc.sync.dma_start(ix[:], idx_d[:, g * NIX:(g + 1) * NIX])
                    for off in range(0, NLO, GCAP):
                        sz = min(GCAP, NLO - off)
                        nc.gpsimd.dma_gather(
                            out_ap=Gt[:, off // 128:(off + sz) // 128, :],
                            in_ap=tabs[0][:],
                            idxs_ap=ix[:, off // 16:(off + sz) // 16],
                            num_idxs=sz, num_idxs_reg=sz,
                            elem_size=row)
                    for off in range(0, NHI, GCAP):
                        sz = min(GCAP, NHI - off)
                        nc.gpsimd.dma_gather(
                            out_ap=Gt[:, gw * k_lo + off // 128:gw * k_lo + (off + sz) // 128, :],
                            in_ap=tabs[1][:],
                            idxs_ap=ix[:, (NLO + off) // 16:(NLO + off + sz) // 16],
                            num_idxs=sz, num_idxs_reg=sz,
                            elem_size=row)
                    # --- one-hot stream (fp8, host-precomputed, [e1 | sm]) ---
                    es = opool.tile([128, 2 * T, 128], f8, tag="es")
                    nc.sync.dma_start(
                        es[:].rearrange("p t j -> p (t j)"),
                        es_d[:, g * 2 * T * 128:(g + 1) * 2 * T * 128])

                    # --- a_d broadcast matmuls ---
                    pad = psad.tile([128, T * nh], f32, tag="pad")
                    for t in range(T):
                        w = w0 + (t // k_lo if t < gw * k_lo else (t - gw * k_lo) // k_hi)
                        nc.tensor.matmul(pad[:, t * nh:(t + 1) * nh],
                                         lhsT=es[:, T + t, :], rhs=adsb[:, w, :],
                                         start=True, stop=True)
                    # --- per-edge scalars: z = a_s + a_d; p = exp(lrelu(z)) ---
                    z = spool.tile([128, T, nh], f32, tag="z")
                    nc.vector.tensor_add(z[:], Gt[:, :, a_s_off:a_s_off + nh],
                                         pad[:].rearrange("p (t h) -> p t h", h=nh))
                    zl = spool.tile([128, T, nh], f32, tag="zl")
                    nc.scalar.activation(zl[:], z[:], AF.Prelu, alpha=NEG_SLOPE)
                    # psc expanded across channels (ACT broadcast-read), f16
                    pexp = spool.tile([128, T, hc], f16, tag="pexp")
                    nc.scalar.activation(
                        pexp[:].rearrange("p t (h c) -> p t h c", h=nh),
                        zl[:].rearrange("p t (h one) -> p t h one", one=1)
                             .broadcast_to([128, T, nh, ch]),
                        AF.Exp)
                    # --- R = [p*h | p] ---
                    R = rpool.tile([128, T, rcols], f16, tag="R")
                    nc.vector.tensor_mul(R[:, :, 0:hc], Gt[:, :, 0:hc], pexp[:])
                    nc.scalar.activation(R[:, :, hc:hc + nh], zl[:], AF.Exp)
                    # --- segment-sum matmuls ---
                    pw = [psW.tile([128, rcols], f32, tag="psW", name=f"pw{layer}_{g}_{wi}")[:]
                          for wi in range(gw)]
                    for t in range(T):
                        if t < gw * k_lo:
                            wi, first = divmod(t, k_lo)
                            is_first = first == 0
                            is_last = (first == k_lo - 1) and k_hi == 0
                        else:
                            wi, r = divmod(t - gw * k_lo, k_hi)
                            is_first = False
                            is_last = r == k_hi - 1
                        nc.tensor.matmul(pw[wi], lhsT=es[:, t, :], rhs=R[:, t, :],
                                         start=is_first, stop=is_last)
                    # --- epilogue per window (DMAs batched per group) ---
                    if layer == 1:
                        rows2 = rowpool.tile([128, gw, 66], f16, tag="rows2")
                    else:
                        og = epool.tile([128, gw, hc], f32, tag="og")
                    for wi in range(gw):
                        w = w0 + wi
                        den = epool.tile([128, nh], f32, tag="den")
                        nc.vector.tensor_scalar_add(den[:], pw[wi][:, hc:hc + nh], EPS)
                        rec = epool.tile([128, nh], f32, tag="rec")
                        nc.vector.reciprocal(rec[:], den[:])
                        if layer == 1:
                            o = epool.tile([128, hc], f32, tag="o")
                            nc.vector.tensor_mul(
                                o[:].rearrange("p (h c) -> p h c", h=heads),
                                pw[wi][:, 0:hc].rearrange("p (h c) -> p h c", h=heads),
                                rec[:].broadcast_to([128, heads, ch]))
                            nc.vector.tensor_add(o[:], o[:], b1_sb[:])
                            # ELU: relu(o) + exp(min(o,0)) - 1
                            r_ = epool.tile([128, hc], f32, tag="relu")
                            nc.scalar.activation(r_[:], o[:], AF.Relu)
                            m_ = epool.tile([128, hc], f32, tag="mneg")
                            nc.vector.tensor_scalar_min(m_[:], o[:], 0.0)
                            nc.scalar.activation(m_[:], m_[:], AF.Exp)
                            act = epool.tile([128, hc], f16, tag="act")
                            nc.vector.scalar_tensor_tensor(
                                act[:], r_[:], -1.0, m_[:], OP.add, OP.add)
                            # transpose into x2T; layer-2 node matmul + table row
                            psT = psTp.tile([128, 128], f16, tag="psT")
                            nc.tensor.transpose(psT[:], act[:], ident[:])
                            nc.vector.tensor_copy(x2T[:, w * 128:(w + 1) * 128], psT[:])
                            ps2 = ps2p.tile([128, 66], f32, tag="ps2")
                            nc.tensor.matmul(ps2[:], lhsT=x2T[:, w * 128:(w + 1) * 128],
                                             rhs=w2e_f16[:], start=True, stop=True)
                            nc.scalar.activation(rows2[:, wi, :], ps2[:], AF.Copy)
                            nc.vector.tensor_copy(adsb2[:, w, :], ps2[:, 65:66])
                        else:
                            nc.vector.scalar_tensor_tensor(
                                og[:, wi, :], pw[wi][:, 0:hc], rec[:], b2_sb[:],
                                OP.mult, OP.add)
                    if layer == 1:
                        ci, ro = wslot(w0)
                        nc.sync.dma_start(
                            t2s[ci][ro:ro + gw * 128, 0:66]
                            .rearrange("(w p) c -> p w c", w=gw), rows2[:])
                        if w0 + gw == W0:
                            ag_chunk(t2s[0], t2f[0])
                        elif w0 + gw == W:
                            ag_chunk(t2s[1], t2f[1])
                    else:
                        nc.sync.dma_start(
                            out_d[w0 * 128:(w0 + gw) * 128, :]
                            .rearrange("(w p) c -> p w c", w=gw), og[:])

            edge_phase(1)
            edge_phase(2)

    nc.compile()
    return nc


# ---------------------------------------------------------------------------
# Entry point
# ---------------------------------------------------------------------------

_CACHE = {}


def _prepare(inputs):
    x = np.ascontiguousarray(np.asarray(inputs["x"], np.float32))
    ei = np.asarray(inputs["edge_index"])
    n_nodes = x.shape[0]
    return pick_config(x, ei, n_nodes)


def _weights_ext(inputs):
    W1 = np.asarray(inputs["W1"], np.float32)
    as1 = np.asarray(inputs["att_src1"], np.float32)
    ad1 = np.asarray(inputs["att_dst1"], np.float32)
    W2 = np.asarray(inputs["W2"], np.float32)
    as2 = np.asarray(inputs["att_src2"], np.float32)
    ad2 = np.asarray(inputs["att_dst2"], np.float32)
    As = np.zeros((HC1, H1), np.float32)
    Ad = np.zeros((HC1, H1), np.float32)
    for h in range(H1):
        As[h * C1:(h + 1) * C1, h] = as1[0, h]
        Ad[h * C1:(h + 1) * C1, h] = ad1[0, h]
    w1e = np.concatenate([W1, W1 @ As, W1 @ Ad], axis=1)           # [128,136]
    w2e = np.concatenate([W2, W2 @ as2[0].T, W2 @ ad2[0].T], axis=1)  # [128,66]
    return np.ascontiguousarray(w1e), np.ascontiguousarray(w2e)


def kernel(**inputs):
    from concourse.bass_utils import run_bass_kernel_spmd

    prep = _prepare(inputs)
    key = (prep["W"], prep["W0"], prep["k_lo"], prep["k_hi"], prep["gw"])
    if key not in _CACHE:
        _CACHE[key] = build_program(dict(
            W=prep["W"], W0=prep["W0"], P=prep["P"], K=prep["K"],
            k_lo=prep["k_lo"], k_hi=prep["k_hi"], gw=prep["gw"]))
    nc = _CACHE[key]

    in_maps = build_in_maps(inputs, prep)
    res = run_bass_kernel_spmd(nc, in_maps, core_ids=list(range(NCORES)))
    return assemble_output(res.results, prep)


def build_in_maps(inputs, prep):
    import concourse.mybir as mybir
    f8np = mybir.dt.np(mybir.dt.float8e4)
    x = np.ascontiguousarray(np.asarray(inputs["x"], np.float32))
    b1 = np.tile(np.asarray(inputs["b1"], np.float32).reshape(1, HC1), (128, 1))
    b2 = np.tile(np.asarray(inputs["b2"], np.float32).reshape(1, C2), (128, 1))
    w1e, w2e = _weights_ext(inputs)
    n_nodes, P, W = prep["n_nodes"], prep["P"], prep["W"]
    po = prep["po"]
    ident = np.eye(128, dtype=np.float16)
    in_maps = []
    # xT_pad per core: columns = core-local padded slots (window-major)
    xT_all = np.zeros((NCORES, IN_CH, P), np.float32)
    node_ids = np.arange(n_nodes)
    c_of = po // P
    col = po % P
    xT_all[c_of, :, col] = x[node_ids]  # fancy: sets [ch] vectors
    for c in range(NCORES):
        m = prep["meta"][c]
        im = dict(
            xT=np.ascontiguousarray(xT_all[c]),
            w1e=w1e, w2e=w2e,
            idx=np.ascontiguousarray(m["idx"]),
            es=m["es"].view(f8np),
            ident=ident,
            b1=b1, b2=b2,
        )
        in_maps.append(im)
    return in_maps


def assemble_output(results, prep):
    full = np.concatenate([results[c]["out"] for c in range(NCORES)], axis=0)
    return np.ascontiguousarray(full[prep["po"]]).astype(np.float32)


# revision 31
# speedup vs baseline: 2.4225x; 1.0064x over previous
"""Trainium2 Bass kernel for a 2-layer GAT encoder (edge-softmax message passing).

Strategy (8 NeuronCores, SPMD single program):
- dst-node partition across cores; host packs each core's dst nodes into
  fixed-count "windows" (<=128 nodes each) and edges into fixed-count
  128-edge tiles per window (K_LO tiles for src in the low half of the
  padded node space, K_HI for the high half -- dma_gather indices are int16).
- Node phase: h_ext = x @ [W | W@att_src | W@att_dst] per core slice,
  fp16 row table written to DRAM in chunks, chunked AllGather pipelined
  with the node matmuls -> full table on every core.
- Edge phase per 128-edge tile: dma_gather rows by src; one-hot matrices
  (fp8, host-precomputed, streamed from DRAM); per-edge a_d via one-hot
  matmul; p = exp(leaky_relu(a_s+a_d)) on the Scalar engine (softmax shift
  invariance makes the segment-max subtraction unnecessary); segment-sum of
  [p*h | p] via one-hot matmul accumulated in PSUM per window; epilogue
  divides and applies bias/ELU. Layer-2 node matmul + table write is
  interleaved per-window into the layer-1 edge phase, with the second
  chunked AllGather riding on top.
- Output rows are window-padded; host de-permutes to the original node order.
"""

import numpy as np

NCORES = 8
HALF = 32768          # int16 gather index limit
ROW1 = 256            # fp16 elems per layer-1 table row (512B): h128|a_s4|a_d4|pad
ROW2 = 128            # fp16 elems per layer-2 table row (256B): h64|a_s|pad
H1, C1 = 4, 32
H2, C2 = 1, 64
IN_CH = 128
HC1 = H1 * C1         # 128
NEG_SLOPE = 0.2
EPS = 1e-16


# ---------------------------------------------------------------------------
# Host-side preprocessing
# ---------------------------------------------------------------------------

def _pack_windows(src, dst, n_nodes, k_lo, k_hi, boundary_arr, per_core,
                  _cache={}):
    """Greedy-pack each core's dst nodes into windows (<=128 nodes, <=k_lo
    lo-tiles, <=k_hi hi-tiles). lo/hi = src node below/above its own core's
    chunk boundary (boundary_arr[core]). Edges must be sorted by dst."""
    key = id(src)
    if _cache.get("key") != key:
        order = np.argsort(dst, kind="stable")
        src_s, dst_s = src[order], dst[order]
        counts = np.bincount(dst_s, minlength=n_nodes)
        starts = np.concatenate([[0], np.cumsum(counts)])
        _cache.update(key=key, src_s=src_s, dst_s=dst_s, starts=starts)
    src_s, dst_s, starts = _cache["src_s"], _cache["dst_s"], _cache["starts"]
    lo_m_s = src_s < boundary_arr[src_s // per_core]
    nlo_n = np.bincount(dst_s[lo_m_s], minlength=n_nodes)
    nall_n = np.bincount(dst_s, minlength=n_nodes)
    nhi_n = nall_n - nlo_n
    cap_lo, cap_hi = k_lo * 128, k_hi * 128
    cores = []
    for c in range(NCORES):
        lo_n, hi_n = c * per_core, (c + 1) * per_core
        bounds = []  # window node ranges [a, b)
        a = lo_n
        cl = ch = cn = 0
        for n in range(lo_n, hi_n):
            el, eh = int(nlo_n[n]), int(nhi_n[n])
            if cn >= 128 or cl + el > cap_lo or ch + eh > cap_hi:
                assert n > a, "single node exceeds tile budget"
                bounds.append((a, n))
                a, cl, ch, cn = n, 0, 0, 0
            cl += el; ch += eh; cn += 1
        bounds.append((a, hi_n))
        wins = []
        for (a, b) in bounds:
            e0, e1_ = starts[a], starts[b]
            es = src_s[e0:e1_]
            dr = (dst_s[e0:e1_] - a).astype(np.int32)
            lm = lo_m_s[e0:e1_]
            wins.append((a, b, es[lm], es[~lm], dr[lm], dr[~lm]))
        cores.append(wins)
    return cores


def _win_edges(src, dst, a, b, n_nodes, per_core):
    c = _pack_windows.__defaults__[0]  # cache dict
    starts, src_s = c["starts"], c["src_s"]
    return src_s[starts[a]:starts[b]]


def _win_tuple(src, dst, a, b, n_nodes, per_core, pi, CH0):
    c = _pack_windows.__defaults__[0]
    starts, src_s, dst_s = c["starts"], c["src_s"], c["dst_s"]
    e0, e1_ = starts[a], starts[b]
    es = src_s[e0:e1_]
    dr = (dst_s[e0:e1_] - a).astype(np.int32)
    lm = pi[es] < CH0
    return (a, b, es[lm], es[~lm], dr[lm], dr[~lm])


def _layout(cores, gw, n_nodes, per_core):
    """Window count, chunk split, and the two permutations.
    pi: node -> global chunk-major table row (chunk 0 = windows [0,W0) of
    every core, chunk 1 = the rest).  po: node -> core-local padded slot
    (c*P + w*128 + s), the output row order."""
    W = max(len(w) for w in cores)
    W = ((W + gw - 1) // gw) * gw
    W0 = ((W // 2 + 1) // 2) * 2  # even: chunk boundary on a group boundary
    P = W * 128
    CH0 = NCORES * W0 * 128
    pi = np.zeros(n_nodes, np.int64)
    po = np.zeros(n_nodes, np.int64)
    boundary_arr = np.full(NCORES, 0, np.int64)
    for c, wins in enumerate(cores):
        boundary_arr[c] = (c + 1) * per_core
        for w, (a, b, *_r) in enumerate(wins):
            ids = np.arange(a, b)
            po[ids] = c * P + w * 128 + (ids - a)
            if w < W0:
                pi[ids] = c * W0 * 128 + w * 128 + (ids - a)
            else:
                pi[ids] = CH0 + c * (W - W0) * 128 + (w - W0) * 128 + (ids - a)
            if w == W0:
                boundary_arr[c] = a
    return W, W0, P, CH0, pi, po, boundary_arr


def host_prep(x, edge_index, n_nodes, k_lo, k_hi, gw):
    """Build the permutations, per-core metadata and index arrays."""
    src = np.ascontiguousarray(edge_index[0]).astype(np.int64)
    dst = np.ascontiguousarray(edge_index[1]).astype(np.int64)
    per_core = n_nodes // NCORES
    assert per_core * NCORES == n_nodes

    # fixpoint on the per-core lo/hi chunk boundaries: the packer's mask only
    # shapes the tile budgets; final buckets use the true pi and re-verify.
    boundary_arr = np.array([c * per_core + per_core // 2 for c in range(NCORES)],
                            np.int64)
    ok = False
    for it in range(14):
        cores = _pack_windows(src, dst, n_nodes, k_lo, k_hi, boundary_arr, per_core)
        W, W0, P, CH0, pi, po, nb = _layout(cores, gw, n_nodes, per_core)
        CH1 = NCORES * (W - W0) * 128
        if CH0 < 32768 and CH1 < 32768:
            # repair: shift tail nodes of overflowing windows (TRUE buckets)
            # into the next window, then re-layout; a few rounds suffice
            for _rep in range(4):
                W, W0, P, CH0, pi, po, _nb2 = _layout(cores, gw, n_nodes, per_core)
                CH1 = NCORES * (W - W0) * 128
                if CH0 >= 32768 or CH1 >= 32768:
                    break
                moved = 0
                bad = False
                for c in range(NCORES):
                    wins = cores[c]
                    nw = []
                    carry = 0  # nodes pushed into the current window from the left
                    for wi_, (a, b, lo_s, hi_s, lo_dr, hi_dr) in enumerate(wins):
                        a -= carry
                        carry = 0
                        while True:
                            es_ = _win_edges(src, dst, a, b, n_nodes, per_core)
                            nlo = int((pi[es_] < CH0).sum())
                            nhi = es_.shape[0] - nlo
                            if (b - a) <= 128 and nlo <= k_lo * 128 and nhi <= k_hi * 128:
                                break
                            if wi_ == len(wins) - 1 or b - a <= 1:
                                bad = True
                                break
                            b -= 1
                            carry += 1
                            moved += 1
                        if bad:
                            break
                        nw.append((a, b))
                    if bad:
                        break
                    cores[c] = [_win_tuple(src, dst, a, b, n_nodes, per_core, pi, CH0)
                                for (a, b) in nw]
                if bad:
                    break
                if moved == 0:
                    ok = True
                    break
            if ok:
                break
        boundary_arr = nb
    assert ok, "chunk-boundary fixpoint failed"
    assert P * NCORES <= 65536

    K = k_lo + k_hi
    meta = []
    G = W // gw
    for c, wins in enumerate(cores):
        idx_lo = np.zeros((W, k_lo * 128), np.int16)
        idx_hi = np.zeros((W, k_hi * 128), np.int16)
        drel = np.full((W, K, 128), 255, np.int32)  # [window, tile-in-window, slot]
        for w, (a, b, lo_s, hi_s, lo_dr, hi_dr) in enumerate(wins):
            es = np.concatenate([lo_s, hi_s])
            dr = np.concatenate([lo_dr, hi_dr])
            lm = pi[es] < CH0                      # true buckets
            pl = pi[es[lm]]
            ph = pi[es[~lm]] - CH0
            ld, hd = dr[lm], dr[~lm]
            assert (pl < CH0).all() and (ph >= 0).all() and (ph < CH1).all()
            idx_lo[w, :len(pl)] = pl.astype(np.int16)
            idx_hi[w, :len(ph)] = ph.astype(np.int16)
            dr_pad = np.full(k_lo * 128, 255, np.int32)
            dr_pad[:len(ld)] = ld
            drel[w, :k_lo] = dr_pad.reshape(k_lo, 128)
            dr_pad = np.full(k_hi * 128, 255, np.int32)
            dr_pad[:len(hd)] = hd
            drel[w, k_lo:] = dr_pad.reshape(k_hi, 128)
        # group-tile order: per group: [lo tiles of gw windows][hi tiles of gw windows]
        tile_order = []  # (window, tile-in-window-index)
        for g in range(G):
            for w in range(g * gw, (g + 1) * gw):
                tile_order += [(w, t) for t in range(k_lo)]
            for w in range(g * gw, (g + 1) * gw):
                tile_order += [(w, k_lo + t) for t in range(k_hi)]
        to = np.array(tile_order)
        drel_t = drel[to[:, 0], to[:, 1]]            # [W*K, 128]
        # one-hot matrices, fp8(e4m3) encoded as raw uint8 bit patterns:
        # 1.0 -> 0x38, 0.0 -> 0x00.
        j = np.arange(128, dtype=np.int32)
        # e1[p, tile, j] = (drel_t[tile, p] == j): edge-slot on partitions
        e1 = (drel_t.T[:, :, None] == j[None, None, :]).astype(np.uint8) * 0x38
        # sm[p, tile, e] = (p == drel_t[tile, e]): dst-rel on partitions
        sm = (j[:, None, None] == drel_t[None, :, :]).astype(np.uint8) * 0x38
        # merged one-hot stream: per group [e1 tiles | sm tiles], fp8 bytes
        T_ = gw * K
        e1g = e1.reshape(128, G, T_, 128)
        smg = sm.reshape(128, G, T_, 128)
        es_ = np.concatenate([e1g, smg], axis=2)         # [128, G, 2T, 128]
        # merged idx stream: per group [lo idxs | hi idxs], wrapped by 16
        def wrap16(a):
            # idx j lives at [j%16, j//16], replicated into all 8 Q7 core
            # partition groups (HW reads each group independently)
            return np.ascontiguousarray(np.tile(a.reshape(-1, 16).T, (8, 1)))
        nlo, nhi = gw * k_lo * 128, gw * k_hi * 128
        ilg = idx_lo.reshape(G, nlo)
        ihg = idx_hi.reshape(G, nhi)
        idx = np.concatenate([ilg, ihg], axis=1)          # [G, nlo+nhi]
        meta.append(dict(
            idx=wrap16(idx),
            e1=np.ascontiguousarray(e1.reshape(128, -1)),
            sm=np.ascontiguousarray(sm.reshape(128, -1)),
            es=np.ascontiguousarray(es_.reshape(128, -1)),
        ))
    return dict(cores=cores, pi=pi, po=po, W=W, W0=W0, P=P, K=K, k_lo=k_lo,
                k_hi=k_hi, gw=gw, meta=meta, n_nodes=n_nodes, per_core=per_core)


def pick_config(x, edge_index, n_nodes):
    """Try candidate (k_lo, k_hi) packings in predicted-cost order; first
    feasible wins. Cost ~ gather bytes + SWDGE call fixed overhead + per-
    window epilogue overhead."""
    gw = 2
    E = edge_index.shape[1]
    lam = E / n_nodes * 128
    base = max(int(np.ceil(lam * 0.5 / 128)), 1)
    cands = []
    for dlo in (-1, 0, 1, 2):
        for dhi in (-1, 0, 1, 2):
            kl, kh = base + dlo, base + dhi
            if kl >= 1 and kh >= 1:
                K_ = kl + kh
                West = int(np.ceil(E / NCORES / 128 / K_ * 1.07 / gw)) * gw
                calls = (West // gw) * (-(-gw * kl * 128 // 1024) + -(-gw * kh * 128 // 1024))
                cost = West * K_ * 50 + calls * 994 + West * 1200
                cands.append((cost, kl, kh))
    cands.sort()
    for _cost, k_lo, k_hi in cands:
        try:
            return host_prep(x, edge_index, n_nodes, k_lo, k_hi, gw)
        except AssertionError:
            continue
    raise AssertionError("no feasible packing found")


# ---------------------------------------------------------------------------
# Bass program
# ---------------------------------------------------------------------------

def build_program(cfg):
    import concourse.bacc as bacc
    import concourse.bass as bass
    import concourse.mybir as mybir
    from concourse import tile

    f32 = mybir.dt.float32
    f16 = mybir.dt.float16
    f8 = mybir.dt.float8e4
    i16 = mybir.dt.int16
    AF = mybir.ActivationFunctionType
    OP = mybir.AluOpType

    W, W0, P, K = cfg["W"], cfg["W0"], cfg["P"], cfg["K"]
    k_lo, k_hi, gw = cfg["k_lo"], cfg["k_hi"], cfg["gw"]
    G = W // gw
    T = gw * K                  # tiles per group
    GCAP = 1024                 # max gather descriptors per SWDGE call (HW limit)
    NLO = gw * k_lo * 128       # lo gather idxs per group
    NHI = gw * k_hi * 128
    W1c = W - W0                # windows in chunk 1
    CH0 = NCORES * W0 * 128     # rows in chunk-0 table
    CH1 = NCORES * W1c * 128

    nc = bacc.Bacc("TRN2", target_bir_lowering=False, debug=False, num_devices=NCORES,
                   dynamic_dma_scratch_size=32768)

    # ---- external inputs ----
    xT = nc.dram_tensor("xT", [IN_CH, P], f32, kind="ExternalInput")
    w1e = nc.dram_tensor("w1e", [IN_CH, 136], f32, kind="ExternalInput")
    w2e = nc.dram_tensor("w2e", [HC1, 66], f32, kind="ExternalInput")
    idx_d = nc.dram_tensor("idx", [128, W * K * 8], i16, kind="ExternalInput")
    es_d = nc.dram_tensor("es", [128, W * K * 256], f8, kind="ExternalInput")
    ident_d = nc.dram_tensor("ident", [128, 128], f16, kind="ExternalInput")
    b1_d = nc.dram_tensor("b1", [128, HC1], f32, kind="ExternalInput")
    b2_d = nc.dram_tensor("b2", [128, C2], f32, kind="ExternalInput")
    out_d = nc.dram_tensor("out", [P, C2], f32, kind="ExternalOutput")

    with tile.TileContext(nc) as tc:
        with (
            tc.tile_pool(name="const", bufs=1) as cpool,
            tc.tile_pool(name="xc", bufs=3) as xcpool,
            tc.tile_pool(name="rows", bufs=3) as rowpool,
            tc.tile_pool(name="gather", bufs=3) as gpool,
            tc.tile_pool(name="onehot", bufs=3) as opool,
            tc.tile_pool(name="rmat", bufs=2) as rpool,
            tc.tile_pool(name="scal", bufs=3) as spool,
            tc.tile_pool(name="idx", bufs=3) as ipool,
            tc.tile_pool(name="epi", bufs=3) as epool,
            tc.tile_pool(name="psA", bufs=1, space="PSUM") as psA,
            tc.tile_pool(name="psT", bufs=1, space="PSUM") as psTp,
            tc.tile_pool(name="ps2", bufs=1, space="PSUM") as ps2p,
            tc.tile_pool(name="psW", bufs=3, space="PSUM") as psW,
            tc.tile_pool(name="psad", bufs=2, space="PSUM") as psad,
            tc.tile_pool(name="dram", bufs=1, space="DRAM") as dpool,
        ):
            # ---- constants to SBUF ----
            w1e_sb = cpool.tile([IN_CH, 136], f32, tag="w1e")
            nc.sync.dma_start(w1e_sb[:], w1e[:])
            w2e_sb = cpool.tile([HC1, 66], f32, tag="w2e")
            nc.sync.dma_start(w2e_sb[:], w2e[:])
            w2e_f16 = cpool.tile([HC1, 66], f16, tag="w2e16")
            nc.vector.tensor_copy(w2e_f16[:], w2e_sb[:])
            ident = cpool.tile([128, 128], f16, tag="ident")
            nc.sync.dma_start(ident[:], ident_d[:])
            b1_sb = cpool.tile([128, HC1], f32, tag="b1")
            nc.sync.dma_start(b1_sb[:], b1_d[:])
            b2_sb = cpool.tile([128, C2], f32, tag="b2")
            nc.sync.dma_start(b2_sb[:], b2_d[:])
            adsb1 = cpool.tile([128, W, H1], f16, tag="adsb1")
            adsb2 = cpool.tile([128, W, H2], f16, tag="adsb2")
            x2T = cpool.tile([128, P], f16, tag="x2T")

            # ---- DRAM tables: 2 chunks, each AllGathered by ONE collective ----
            t1s = [dpool.tile([W0 * 128, ROW1], f16, tag="t1s0", name="t1s0"),
                   dpool.tile([W1c * 128, ROW1], f16, tag="t1s1", name="t1s1")]
            t1f = [dpool.tile([CH0, ROW1], f16, tag="t1f0", name="t1f0", addr_space="Shared"),
                   dpool.tile([CH1, ROW1], f16, tag="t1f1", name="t1f1", addr_space="Shared")]
            t2s = [dpool.tile([W0 * 128, ROW2], f16, tag="t2s0", name="t2s0"),
                   dpool.tile([W1c * 128, ROW2], f16, tag="t2s1", name="t2s1")]
            t2f = [dpool.tile([CH0, ROW2], f16, tag="t2f0", name="t2f0", addr_space="Shared"),
                   dpool.tile([CH1, ROW2], f16, tag="t2f1", name="t2f1", addr_space="Shared")]

            def ag_chunk(slice_t, full_t):
                nc.gpsimd.collective_compute(
                    "AllGather", mybir.AluOpType.bypass,
                    replica_groups=[list(range(NCORES))],
                    ins=[slice_t.opt()], outs=[full_t.opt()],
                )

            def wslot(w):
                # (chunk index, row offset of window w in its chunk slice)
                return (0, w * 128) if w < W0 else (1, (w - W0) * 128)

            # ================= phase A: layer-1 node matmul =================
            # batches of 4 windows per xc load / rows store (HWDGE-bound phase)
            assert W % 2 == 0 and W0 % 2 == 0
            wb = 0
            while wb < W:
                nw = min(4, W - wb, (W0 if wb < W0 else W) - wb)
                xc = xcpool.tile([IN_CH, 4 * 128], f32, tag="xc")
                nc.sync.dma_start(xc[:, 0:nw * 128],
                                  xT[:, wb * 128:(wb + nw) * 128])
                rows = rowpool.tile([128, 4, 136], f16, tag="rows1")
                for j in range(nw):
                    w = wb + j
                    ps = psA.tile([128, 136], f32, tag="ps_node")
                    nc.tensor.matmul(ps[:], lhsT=xc[:, j * 128:(j + 1) * 128],
                                     rhs=w1e_sb[:], start=True, stop=True)
                    nc.scalar.activation(rows[:, j, :], ps[:], AF.Copy)
                    nc.vector.tensor_copy(adsb1[:, w, :], ps[:, 132:136])
                ci, ro = wslot(wb)
                nc.sync.dma_start(
                    t1s[ci][ro:ro + nw * 128, 0:136]
                    .rearrange("(w p) c -> p w c", w=nw), rows[:, 0:nw, :])
                wb += nw
                if wb == W0:
                    ag_chunk(t1s[0], t1f[0])
                elif wb == W:
                    ag_chunk(t1s[1], t1f[1])

            # ================= edge phase (shared for both layers) ==========
            def edge_phase(layer):
                if layer == 1:
                    tabs, row, heads, ch = t1f, ROW1, H1, C1
                    adsb = adsb1
                    a_s_off = 128
                    rcols = HC1 + H1  # 132
                else:
                    tabs, row, heads, ch = t2f, ROW2, H2, C2
                    adsb = adsb2
                    a_s_off = 64
                    rcols = C2 + H2  # 65
                hc = heads * ch
                nh = heads  # scalar cols per tile
                NIX = (NLO + NHI) // 16
                for g in range(G):
                    w0 = g * gw
                    # --- gather (merged idx load: [lo | hi] per group) ---
                    Gt = gpool.tile([128, T, row], f16, tag=f"G{layer}")
                    ix = ipool.tile([128, NIX], i16, tag="ix")
                    nc.sync.dma_start(ix[:], idx_d[:, g * NIX:(g + 1) * NIX])
                    for off in range(0, NLO, GCAP):
                        sz = min(GCAP, NLO - off)
                        nc.gpsimd.dma_gather(
                            out_ap=Gt[:, off // 128:(off + sz) // 128, :],
                            in_ap=tabs[0][:],
                            idxs_ap=ix[:, off // 16:(off + sz) // 16],
                            num_idxs=sz, num_idxs_reg=sz,
                            elem_size=row)
                    for off in range(0, NHI, GCAP):
                        sz = min(GCAP, NHI - off)
                        nc.gpsimd.dma_gather(
                            out_ap=Gt[:, gw * k_lo + off // 128:gw * k_lo + (off + sz) // 128, :],
                            in_ap=tabs[1][:],
                            idxs_ap=ix[:, (NLO + off) // 16:(NLO + off + sz) // 16],
                            num_idxs=sz, num_idxs_reg=sz,
                            elem_size=row)
                    # --- one-hot stream (fp8, host-precomputed, [e1 | sm]) ---
                    es = opool.tile([128, 2 * T, 128], f8, tag="es")
                    nc.sync.dma_start(
                        es[:].rearrange("p t j -> p (t j)"),
                        es_d[:, g * 2 * T * 128:(g + 1) * 2 * T * 128])

                    # --- a_d broadcast matmuls ---
                    pad = psad.tile([128, T * nh], f32, tag="pad")
                    for t in range(T):
                        w = w0 + (t // k_lo if t < gw * k_lo else (t - gw * k_lo) // k_hi)
                        nc.tensor.matmul(pad[:, t * nh:(t + 1) * nh],
                                         lhsT=es[:, T + t, :], rhs=adsb[:, w, :],
                                         start=True, stop=True)
                    # --- per-edge scalars: z = a_s + a_d; p = exp(lrelu(z)) ---
                    z = spool.tile([128, T, nh], f32, tag="z")
                    nc.vector.tensor_add(z[:], Gt[:, :, a_s_off:a_s_off + nh],
                                         pad[:].rearrange("p (t h) -> p t h", h=nh))
                    zl = spool.tile([128, T, nh], f32, tag="zl")
                    nc.scalar.activation(zl[:], z[:], AF.Prelu, alpha=NEG_SLOPE)
                    # psc expanded across channels (ACT broadcast-read), f16
                    pexp = spool.tile([128, T, hc], f16, tag="pexp")
                    nc.scalar.activation(
                        pexp[:].rearrange("p t (h c) -> p t h c", h=nh),
                        zl[:].rearrange("p t (h one) -> p t h one", one=1)
                             .broadcast_to([128, T, nh, ch]),
                        AF.Exp)
                    # --- R = [p*h | p] ---
                    R = rpool.tile([128, T, rcols], f16, tag="R")
                    nc.vector.tensor_mul(R[:, :, 0:hc], Gt[:, :, 0:hc], pexp[:])
                    nc.scalar.activation(R[:, :, hc:hc + nh], zl[:], AF.Exp)
                    # --- segment-sum matmuls ---
                    pw = [psW.tile([128, rcols], f32, tag="psW", name=f"pw{layer}_{g}_{wi}")[:]
                          for wi in range(gw)]
                    for t in range(T):
                        if t < gw * k_lo:
                            wi, first = divmod(t, k_lo)
                            is_first = first == 0
                            is_last = (first == k_lo - 1) and k_hi == 0
                        else:
                            wi, r = divmod(t - gw * k_lo, k_hi)
                            is_first = False
                            is_last = r == k_hi - 1
                        nc.tensor.matmul(pw[wi], lhsT=es[:, t, :], rhs=R[:, t, :],
                                         start=is_first, stop=is_last)
                    # --- epilogue per window (DMAs batched per group) ---
                    if layer == 1:
                        rows2 = rowpool.tile([128, gw, 66], f16, tag="rows2")
                    else:
                        og = epool.tile([128, gw, hc], f32, tag="og")
                    for wi in range(gw):
                        w = w0 + wi
                        den = epool.tile([128, nh], f32, tag="den")
                        nc.vector.tensor_scalar_add(den[:], pw[wi][:, hc:hc + nh], EPS)
                        rec = epool.tile([128, nh], f32, tag="rec")
                        nc.vector.reciprocal(rec[:], den[:])
                        if layer == 1:
                            o = epool.tile([128, hc], f32, tag="o")
                            nc.vector.tensor_mul(
                                o[:].rearrange("p (h c) -> p h c", h=heads),
                                pw[wi][:, 0:hc].rearrange("p (h c) -> p h c", h=heads),
                                rec[:].broadcast_to([128, heads, ch]))
                            nc.vector.tensor_add(o[:], o[:], b1_sb[:])
                            # ELU: relu(o) + exp(min(o,0)) - 1
                            r_ = epool.tile([128, hc], f32, tag="relu")
                            nc.scalar.activation(r_[:], o[:], AF.Relu)
                            m_ = epool.tile([128, hc], f32, tag="mneg")
                            nc.vector.tensor_scalar_min(m_[:], o[:], 0.0)
                            nc.scalar.activation(m_[:], m_[:], AF.Exp)
                            act = epool.tile([128, hc], f16, tag="act")
                            nc.vector.scalar_tensor_tensor(
                                act[:], r_[:], -1.0, m_[:], OP.add, OP.add)
                            # transpose into x2T; layer-2 node matmul + table row
                            psT = psTp.tile([128, 128], f16, tag="psT")
                            nc.tensor.transpose(psT[:], act[:], ident[:])
                            nc.vector.tensor_copy(x2T[:, w * 128:(w + 1) * 128], psT[:])
                            ps2 = ps2p.tile([128, 66], f32, tag="ps2")
                            nc.tensor.matmul(ps2[:], lhsT=x2T[:, w * 128:(w + 1) * 128],
                                             rhs=w2e_f16[:], start=True, stop=True)
                            nc.scalar.activation(rows2[:, wi, :], ps2[:], AF.Copy)
                            nc.vector.tensor_copy(adsb2[:, w, :], ps2[:, 65:66])
                        else:
                            nc.vector.scalar_tensor_tensor(
                                og[:, wi, :], pw[wi][:, 0:hc], rec[:], b2_sb[:],
                                OP.mult, OP.add)
                    if layer == 1:
                        ci, ro = wslot(w0)
                        nc.sync.dma_start(
                            t2s[ci][ro:ro + gw * 128, 0:66]
                            .rearrange("(w p) c -> p w c", w=gw), rows2[:])
                        if w0 + gw == W0:
                            ag_chunk(t2s[0], t2f[0])
                        elif w0 + gw == W:
                            ag_chunk(t2s[1], t2f[1])
                    else:
                        nc.sync.dma_start(
                            out_d[w0 * 128:(w0 + gw) * 128, :]
                            .rearrange("(w p) c -> p w c", w=gw), og[:])

            edge_phase(1)
            edge_phase(2)

    nc.compile()
    return nc


# ---------------------------------------------------------------------------
# Entry point
# ---------------------------------------------------------------------------

_CACHE = {}


def _prepare(inputs):
    x = np.ascontiguousarray(np.asarray(inputs["x"], np.float32))
    ei = np.asarray(inputs["edge_index"])
    n_nodes = x.shape[0]
    return pick_config(x, ei, n_nodes)


def _weights_ext(inputs):
    W1 = np.asarray(inputs["W1"], np.float32)
    as1 = np.asarray(inputs["att_src1"], np.float32)
    ad1 = np.asarray(inputs["att_dst1"], np.float32)
    W2 = np.asarray(inputs["W2"], np.float32)
    as2 = np.asarray(inputs["att_src2"], np.float32)
    ad2 = np.asarray(inputs["att_dst2"], np.float32)
    As = np.zeros((HC1, H1), np.float32)
    Ad = np.zeros((HC1, H1), np.float32)
    for h in range(H1):
        As[h * C1:(h + 1) * C1, h] = as1[0, h]
        Ad[h * C1:(h + 1) * C1, h] = ad1[0, h]
    w1e = np.concatenate([W1, W1 @ As, W1 @ Ad], axis=1)           # [128,136]
    w2e = np.concatenate([W2, W2 @ as2[0].T, W2 @ ad2[0].T], axis=1)  # [128,66]
    return np.ascontiguousarray(w1e), np.ascontiguousarray(w2e)


def kernel(**inputs):
    from concourse.bass_utils import run_bass_kernel_spmd

    prep = _prepare(inputs)
    key = (prep["W"], prep["W0"], prep["k_lo"], prep["k_hi"], prep["gw"])
    if key not in _CACHE:
        _CACHE[key] = build_program(dict(
            W=prep["W"], W0=prep["W0"], P=prep["P"], K=prep["K"],
            k_lo=prep["k_lo"], k_hi=prep["k_hi"], gw=prep["gw"]))
    nc = _CACHE[key]

    in_maps = build_in_maps(inputs, prep)
    res = run_bass_kernel_spmd(nc, in_maps, core_ids=list(range(NCORES)))
    return assemble_output(res.results, prep)


def build_in_maps(inputs, prep):
    import concourse.mybir as mybir
    f8np = mybir.dt.np(mybir.dt.float8e4)
    x = np.ascontiguousarray(np.asarray(inputs["x"], np.float32))
    b1 = np.tile(np.asarray(inputs["b1"], np.float32).reshape(1, HC1), (128, 1))
    b2 = np.tile(np.asarray(inputs["b2"], np.float32).reshape(1, C2), (128, 1))
    w1e, w2e = _weights_ext(inputs)
    n_nodes, P, W = prep["n_nodes"], prep["P"], prep["W"]
    po = prep["po"]
    ident = np.eye(128, dtype=np.float16)
    in_maps = []
    # xT_pad per core: columns = core-local padded slots (window-major)
    xT_all = np.zeros((NCORES, IN_CH, P), np.float32)
    node_ids = np.arange(n_nodes)
    c_of = po // P
    col = po % P
    xT_all[c_of, :, col] = x[node_ids]  # fancy: sets [ch] vectors
    for c in range(NCORES):
        m = prep["meta"][c]
        im = dict(
            xT=np.ascontiguousarray(xT_all[c]),
            w1e=w1e, w2e=w2e,
            idx=np.ascontiguousarray(m["idx"]),
            es=m["es"].view(f8np),
            ident=ident,
            b1=b1, b2=b2,
        )
        in_maps.append(im)
    return in_maps


def assemble_output(results, prep):
    full = np.concatenate([results[c]["out"] for c in range(NCORES)], axis=0)
    return np.ascontiguousarray(full[prep["po"]]).astype(np.float32)


# revision 33
# speedup vs baseline: 3.3465x; 1.3814x over previous
"""Trainium2 Bass kernel for a 2-layer GAT encoder (edge-softmax message passing).

Strategy (8 NeuronCores, SPMD single program):
- dst-node partition across cores; host packs each core's dst nodes into
  fixed-count "windows" (<=128 nodes each) and edges into fixed-count
  128-edge tiles per window (K_LO tiles for src in the low half of the
  padded node space, K_HI for the high half -- dma_gather indices are int16).
- Node phase: h_ext = x @ [W | W@att_src | W@att_dst] per core slice,
  fp16 row table written to DRAM in chunks, chunked AllGather pipelined
  with the node matmuls -> full table on every core.
- Edge phase per 128-edge tile: dma_gather rows by src; one-hot matrices
  (fp8, host-precomputed, streamed from DRAM); per-edge a_d via one-hot
  matmul; p = exp(leaky_relu(a_s+a_d)) on the Scalar engine (softmax shift
  invariance makes the segment-max subtraction unnecessary); segment-sum of
  [p*h | p] via one-hot matmul accumulated in PSUM per window; epilogue
  divides and applies bias/ELU. Layer-2 node matmul + table write is
  interleaved per-window into the layer-1 edge phase, with the second
  chunked AllGather riding on top.
- Output rows are window-padded; host de-permutes to the original node order.
"""

import numpy as np

NCORES = 8
HALF = 32768          # int16 gather index limit
ROW1 = 256            # fp16 elems per layer-1 table row (512B): h128|a_s4|a_d4|pad
ROW2 = 128            # fp16 elems per layer-2 table row (256B): h64|a_s|pad
H1, C1 = 4, 32
H2, C2 = 1, 64
IN_CH = 128
HC1 = H1 * C1         # 128
NEG_SLOPE = 0.2
EPS = 1e-16


# ---------------------------------------------------------------------------
# Host-side preprocessing
# ---------------------------------------------------------------------------

def _pack_windows(src, dst, n_nodes, k_lo, k_hi, boundary_arr, per_core,
                  _cache={}):
    """Greedy-pack each core's dst nodes into windows (<=128 nodes, <=k_lo
    lo-tiles, <=k_hi hi-tiles). lo/hi = src node below/above its own core's
    chunk boundary (boundary_arr[core]). Edges must be sorted by dst."""
    key = id(src)
    if _cache.get("key") != key:
        order = np.argsort(dst, kind="stable")
        src_s, dst_s = src[order], dst[order]
        counts = np.bincount(dst_s, minlength=n_nodes)
        starts = np.concatenate([[0], np.cumsum(counts)])
        _cache.update(key=key, src_s=src_s, dst_s=dst_s, starts=starts)
    src_s, dst_s, starts = _cache["src_s"], _cache["dst_s"], _cache["starts"]
    lo_m_s = src_s < boundary_arr[src_s // per_core]
    nlo_n = np.bincount(dst_s[lo_m_s], minlength=n_nodes)
    nall_n = np.bincount(dst_s, minlength=n_nodes)
    nhi_n = nall_n - nlo_n
    cap_lo, cap_hi = k_lo * 128, k_hi * 128
    cores = []
    for c in range(NCORES):
        lo_n, hi_n = c * per_core, (c + 1) * per_core
        bounds = []  # window node ranges [a, b)
        a = lo_n
        cl = ch = cn = 0
        for n in range(lo_n, hi_n):
            el, eh = int(nlo_n[n]), int(nhi_n[n])
            if cn >= 128 or cl + el > cap_lo or ch + eh > cap_hi:
                assert n > a, "single node exceeds tile budget"
                bounds.append((a, n))
                a, cl, ch, cn = n, 0, 0, 0
            cl += el; ch += eh; cn += 1
        bounds.append((a, hi_n))
        wins = []
        for (a, b) in bounds:
            e0, e1_ = starts[a], starts[b]
            es = src_s[e0:e1_]
            dr = (dst_s[e0:e1_] - a).astype(np.int32)
            lm = lo_m_s[e0:e1_]
            wins.append((a, b, es[lm], es[~lm], dr[lm], dr[~lm]))
        cores.append(wins)
    return cores


def _win_edges(src, dst, a, b, n_nodes, per_core):
    c = _pack_windows.__defaults__[0]  # cache dict
    starts, src_s = c["starts"], c["src_s"]
    return src_s[starts[a]:starts[b]]


def _win_tuple(src, dst, a, b, n_nodes, per_core, pi, CH0):
    c = _pack_windows.__defaults__[0]
    starts, src_s, dst_s = c["starts"], c["src_s"], c["dst_s"]
    e0, e1_ = starts[a], starts[b]
    es = src_s[e0:e1_]
    dr = (dst_s[e0:e1_] - a).astype(np.int32)
    lm = pi[es] < CH0
    return (a, b, es[lm], es[~lm], dr[lm], dr[~lm])


def _layout(cores, gw, n_nodes, per_core):
    """Window count, chunk split, and the two permutations.
    pi: node -> global chunk-major table row (chunk 0 = windows [0,W0) of
    every core, chunk 1 = the rest).  po: node -> core-local padded slot
    (c*P + w*128 + s), the output row order."""
    W = max(len(w) for w in cores)
    W = ((W + gw - 1) // gw) * gw
    W0 = ((W // 2 + 1) // 2) * 2  # even: chunk boundary on a group boundary
    P = W * 128
    CH0 = NCORES * W0 * 128
    pi = np.zeros(n_nodes, np.int64)
    po = np.zeros(n_nodes, np.int64)
    boundary_arr = np.full(NCORES, 0, np.int64)
    for c, wins in enumerate(cores):
        boundary_arr[c] = (c + 1) * per_core
        for w, (a, b, *_r) in enumerate(wins):
            ids = np.arange(a, b)
            po[ids] = c * P + w * 128 + (ids - a)
            if w < W0:
                pi[ids] = c * W0 * 128 + w * 128 + (ids - a)
            else:
                pi[ids] = CH0 + c * (W - W0) * 128 + (w - W0) * 128 + (ids - a)
            if w == W0:
                boundary_arr[c] = a
    return W, W0, P, CH0, pi, po, boundary_arr


def host_prep(x, edge_index, n_nodes, k_lo, k_hi, gw):
    """Build the permutations, per-core metadata and index arrays."""
    src = np.ascontiguousarray(edge_index[0]).astype(np.int64)
    dst = np.ascontiguousarray(edge_index[1]).astype(np.int64)
    per_core = n_nodes // NCORES
    assert per_core * NCORES == n_nodes

    # fixpoint on the per-core lo/hi chunk boundaries: the packer's mask only
    # shapes the tile budgets; final buckets use the true pi and re-verify.
    boundary_arr = np.array([c * per_core + per_core // 2 for c in range(NCORES)],
                            np.int64)
    ok = False
    for it in range(14):
        cores = _pack_windows(src, dst, n_nodes, k_lo, k_hi, boundary_arr, per_core)
        W, W0, P, CH0, pi, po, nb = _layout(cores, gw, n_nodes, per_core)
        CH1 = NCORES * (W - W0) * 128
        if CH0 < 32768 and CH1 < 32768:
            # repair: shift tail nodes of overflowing windows (TRUE buckets)
            # into the next window, then re-layout; a few rounds suffice
            for _rep in range(4):
                W, W0, P, CH0, pi, po, _nb2 = _layout(cores, gw, n_nodes, per_core)
                CH1 = NCORES * (W - W0) * 128
                if CH0 >= 32768 or CH1 >= 32768:
                    break
                moved = 0
                bad = False
                for c in range(NCORES):
                    wins = cores[c]
                    nw = []
                    carry = 0  # nodes pushed into the current window from the left
                    for wi_, (a, b, lo_s, hi_s, lo_dr, hi_dr) in enumerate(wins):
                        a -= carry
                        carry = 0
                        while True:
                            es_ = _win_edges(src, dst, a, b, n_nodes, per_core)
                            nlo = int((pi[es_] < CH0).sum())
                            nhi = es_.shape[0] - nlo
                            if (b - a) <= 128 and nlo <= k_lo * 128 and nhi <= k_hi * 128:
                                break
                            if wi_ == len(wins) - 1 or b - a <= 1:
                                bad = True
                                break
                            b -= 1
                            carry += 1
                            moved += 1
                        if bad:
                            break
                        nw.append((a, b))
                    if bad:
                        break
                    cores[c] = [_win_tuple(src, dst, a, b, n_nodes, per_core, pi, CH0)
                                for (a, b) in nw]
                if bad:
                    break
                if moved == 0:
                    ok = True
                    break
            if ok:
                break
        boundary_arr = nb
    assert ok, "chunk-boundary fixpoint failed"
    assert P * NCORES <= 65536

    K = k_lo + k_hi
    meta = []
    G = W // gw
    for c, wins in enumerate(cores):
        idx_lo = np.zeros((W, k_lo * 128), np.int16)
        idx_hi = np.zeros((W, k_hi * 128), np.int16)
        drel = np.full((W, K, 128), 255, np.int32)  # [window, tile-in-window, slot]
        for w, (a, b, lo_s, hi_s, lo_dr, hi_dr) in enumerate(wins):
            es = np.concatenate([lo_s, hi_s])
            dr = np.concatenate([lo_dr, hi_dr])
            lm = pi[es] < CH0                      # true buckets
            pl = pi[es[lm]]
            ph = pi[es[~lm]] - CH0
            ld, hd = dr[lm], dr[~lm]
            assert (pl < CH0).all() and (ph >= 0).all() and (ph < CH1).all()
            idx_lo[w, :len(pl)] = pl.astype(np.int16)
            idx_hi[w, :len(ph)] = ph.astype(np.int16)
            dr_pad = np.full(k_lo * 128, 255, np.int32)
            dr_pad[:len(ld)] = ld
            drel[w, :k_lo] = dr_pad.reshape(k_lo, 128)
            dr_pad = np.full(k_hi * 128, 255, np.int32)
            dr_pad[:len(hd)] = hd
            drel[w, k_lo:] = dr_pad.reshape(k_hi, 128)
        # group-tile order: per group: [lo tiles of gw windows][hi tiles of gw windows]
        tile_order = []  # (window, tile-in-window-index)
        for g in range(G):
            for w in range(g * gw, (g + 1) * gw):
                tile_order += [(w, t) for t in range(k_lo)]
            for w in range(g * gw, (g + 1) * gw):
                tile_order += [(w, k_lo + t) for t in range(k_hi)]
        to = np.array(tile_order)
        drel_t = drel[to[:, 0], to[:, 1]]            # [W*K, 128]
        # one-hot matrices, fp8(e4m3) encoded as raw uint8 bit patterns:
        # 1.0 -> 0x38, 0.0 -> 0x00.
        j = np.arange(128, dtype=np.int32)
        # e1[p, tile, j] = (drel_t[tile, p] == j): edge-slot on partitions
        e1 = (drel_t.T[:, :, None] == j[None, None, :]).astype(np.uint8) * 0x38
        # sm[p, tile, e] = (p == drel_t[tile, e]): dst-rel on partitions
        sm = (j[:, None, None] == drel_t[None, :, :]).astype(np.uint8) * 0x38
        # merged one-hot stream: per group [e1 tiles | sm tiles], fp8 bytes
        T_ = gw * K
        e1g = e1.reshape(128, G, T_, 128)
        smg = sm.reshape(128, G, T_, 128)
        es_ = np.concatenate([e1g, smg], axis=2)         # [128, G, 2T, 128]
        # merged idx stream: per group [lo idxs | hi idxs], wrapped by 16
        def wrap16(a):
            # idx j lives at [j%16, j//16], replicated into all 8 Q7 core
            # partition groups (HW reads each group independently)
            return np.ascontiguousarray(np.tile(a.reshape(-1, 16).T, (8, 1)))
        nlo, nhi = gw * k_lo * 128, gw * k_hi * 128
        ilg = idx_lo.reshape(G, nlo)
        ihg = idx_hi.reshape(G, nhi)
        idx = np.concatenate([ilg, ihg], axis=1)          # [G, nlo+nhi]
        meta.append(dict(
            idx=wrap16(idx),
            e1=np.ascontiguousarray(e1.reshape(128, -1)),
            sm=np.ascontiguousarray(sm.reshape(128, -1)),
            es=np.ascontiguousarray(es_.reshape(128, -1)),
            dc=np.ascontiguousarray(
                np.minimum(drel_t.T, 255).astype(np.uint8)),
        ))
    return dict(cores=cores, pi=pi, po=po, W=W, W0=W0, P=P, K=K, k_lo=k_lo,
                k_hi=k_hi, gw=gw, meta=meta, n_nodes=n_nodes, per_core=per_core)


def pick_config(x, edge_index, n_nodes):
    """Try candidate (k_lo, k_hi) packings in predicted-cost order; first
    feasible wins. Cost ~ gather bytes + SWDGE call fixed overhead + per-
    window epilogue overhead."""
    gw = 2
    E = edge_index.shape[1]
    lam = E / n_nodes * 128
    base = max(int(np.ceil(lam * 0.5 / 128)), 1)
    cands = []
    for dlo in (-1, 0, 1, 2):
        for dhi in (-1, 0, 1, 2):
            kl, kh = base + dlo, base + dhi
            if kl >= 1 and kh >= 1:
                K_ = kl + kh
                West = int(np.ceil(E / NCORES / 128 / K_ * 1.07 / gw)) * gw
                calls = (West // gw) * (-(-gw * kl * 128 // 1024) + -(-gw * kh * 128 // 1024))
                cost = West * K_ * 50 + calls * 994 + West * 1200
                cands.append((cost, kl, kh))
    cands.sort()
    for _cost, k_lo, k_hi in cands:
        try:
            return host_prep(x, edge_index, n_nodes, k_lo, k_hi, gw)
        except AssertionError:
            continue
    raise AssertionError("no feasible packing found")


# ---------------------------------------------------------------------------
# Bass program
# ---------------------------------------------------------------------------

def build_program(cfg):
    import concourse.bacc as bacc
    import concourse.bass as bass
    import concourse.mybir as mybir
    from concourse import tile

    f32 = mybir.dt.float32
    f16 = mybir.dt.float16
    f8 = mybir.dt.float8e4
    i16 = mybir.dt.int16
    AF = mybir.ActivationFunctionType
    OP = mybir.AluOpType

    W, W0, P, K = cfg["W"], cfg["W0"], cfg["P"], cfg["K"]
    k_lo, k_hi, gw = cfg["k_lo"], cfg["k_hi"], cfg["gw"]
    G = W // gw
    T = gw * K                  # tiles per group
    GCAP = 1024                 # max gather descriptors per SWDGE call (HW limit)
    NLO = gw * k_lo * 128       # lo gather idxs per group
    NHI = gw * k_hi * 128
    W1c = W - W0                # windows in chunk 1
    CH0 = NCORES * W0 * 128     # rows in chunk-0 table
    CH1 = NCORES * W1c * 128

    nc = bacc.Bacc("TRN2", target_bir_lowering=False, debug=False, num_devices=NCORES,
                   dynamic_dma_scratch_size=32768)

    # ---- external inputs ----
    xT = nc.dram_tensor("xT", [IN_CH, P], f32, kind="ExternalInput")
    w1e = nc.dram_tensor("w1e", [IN_CH, 136], f32, kind="ExternalInput")
    w2e = nc.dram_tensor("w2e", [HC1, 66], f32, kind="ExternalInput")
    u8 = mybir.dt.uint8
    idx_d = nc.dram_tensor("idx", [128, W * K * 8], i16, kind="ExternalInput")
    es_d = nc.dram_tensor("es", [128, W * K * 256], f8, kind="ExternalInput")
    dc_d = nc.dram_tensor("dc", [128, W * K], u8, kind="ExternalInput")
    iota_r_d = nc.dram_tensor("iota_r", [128, 128], u8, kind="ExternalInput")
    ident_d = nc.dram_tensor("ident", [128, 128], f16, kind="ExternalInput")
    b1_d = nc.dram_tensor("b1", [128, HC1], f32, kind="ExternalInput")
    b2_d = nc.dram_tensor("b2", [128, C2], f32, kind="ExternalInput")
    out_d = nc.dram_tensor("out", [P, C2], f32, kind="ExternalOutput")

    with tile.TileContext(nc) as tc:
        with (
            tc.tile_pool(name="const", bufs=1) as cpool,
            tc.tile_pool(name="xc", bufs=3) as xcpool,
            tc.tile_pool(name="rows", bufs=3) as rowpool,
            tc.tile_pool(name="gather", bufs=3) as gpool,
            tc.tile_pool(name="onehot", bufs=3) as opool,
            tc.tile_pool(name="onehot2", bufs=2) as opool2,
            tc.tile_pool(name="rmat", bufs=2) as rpool,
            tc.tile_pool(name="scal", bufs=3) as spool,
            tc.tile_pool(name="idx", bufs=3) as ipool,
            tc.tile_pool(name="epi", bufs=3) as epool,
            tc.tile_pool(name="psA", bufs=1, space="PSUM") as psA,
            tc.tile_pool(name="psT", bufs=1, space="PSUM") as psTp,
            tc.tile_pool(name="ps2", bufs=1, space="PSUM") as ps2p,
            tc.tile_pool(name="psW", bufs=3, space="PSUM") as psW,
            tc.tile_pool(name="psad", bufs=2, space="PSUM") as psad,
            tc.tile_pool(name="dram", bufs=1, space="DRAM") as dpool,
        ):
            # ---- constants to SBUF ----
            w1e_sb = cpool.tile([IN_CH, 136], f32, tag="w1e")
            nc.sync.dma_start(w1e_sb[:], w1e[:])
            w2e_sb = cpool.tile([HC1, 66], f32, tag="w2e")
            nc.sync.dma_start(w2e_sb[:], w2e[:])
            w2e_f16 = cpool.tile([HC1, 66], f16, tag="w2e16")
            nc.vector.tensor_copy(w2e_f16[:], w2e_sb[:])
            ident = cpool.tile([128, 128], f16, tag="ident")
            nc.sync.dma_start(ident[:], ident_d[:])
            iota_r = cpool.tile([128, 128], u8, tag="iota_r")
            nc.sync.dma_start(iota_r[:], iota_r_d[:])
            b1_sb = cpool.tile([128, HC1], f32, tag="b1")
            nc.sync.dma_start(b1_sb[:], b1_d[:])
            b2_sb = cpool.tile([128, C2], f32, tag="b2")
            nc.sync.dma_start(b2_sb[:], b2_d[:])
            adsb1 = cpool.tile([128, W, H1], f16, tag="adsb1")
            adsb2 = cpool.tile([128, W, H2], f16, tag="adsb2")
            x2T = cpool.tile([128, P], f16, tag="x2T")

            # ---- DRAM tables: 2 chunks, each AllGathered by ONE collective ----
            t1s = [dpool.tile([W0 * 128, ROW1], f16, tag="t1s0", name="t1s0"),
                   dpool.tile([W1c * 128, ROW1], f16, tag="t1s1", name="t1s1")]
            t1f = [dpool.tile([CH0, ROW1], f16, tag="t1f0", name="t1f0", addr_space="Shared"),
                   dpool.tile([CH1, ROW1], f16, tag="t1f1", name="t1f1", addr_space="Shared")]
            t2s = [dpool.tile([W0 * 128, ROW2], f16, tag="t2s0", name="t2s0"),
                   dpool.tile([W1c * 128, ROW2], f16, tag="t2s1", name="t2s1")]
            t2f = [dpool.tile([CH0, ROW2], f16, tag="t2f0", name="t2f0", addr_space="Shared"),
                   dpool.tile([CH1, ROW2], f16, tag="t2f1", name="t2f1", addr_space="Shared")]

            def ag_chunk(slice_t, full_t):
                nc.gpsimd.collective_compute(
                    "AllGather", mybir.AluOpType.bypass,
                    replica_groups=[list(range(NCORES))],
                    ins=[slice_t.opt()], outs=[full_t.opt()],
                )

            def wslot(w):
                # (chunk index, row offset of window w in its chunk slice)
                return (0, w * 128) if w < W0 else (1, (w - W0) * 128)

            # ================= phase A: layer-1 node matmul =================
            # batches of 4 windows per xc load / rows store (HWDGE-bound phase)
            assert W % 2 == 0 and W0 % 2 == 0
            wb = 0
            while wb < W:
                nw = min(4, W - wb, (W0 if wb < W0 else W) - wb)
                xc = xcpool.tile([IN_CH, 4 * 128], f32, tag="xc")
                nc.sync.dma_start(xc[:, 0:nw * 128],
                                  xT[:, wb * 128:(wb + nw) * 128])
                rows = rowpool.tile([128, 4, 136], f16, tag="rows1")
                for j in range(nw):
                    w = wb + j
                    ps = psA.tile([128, 136], f32, tag="ps_node")
                    nc.tensor.matmul(ps[:], lhsT=xc[:, j * 128:(j + 1) * 128],
                                     rhs=w1e_sb[:], start=True, stop=True)
                    nc.scalar.activation(rows[:, j, :], ps[:], AF.Copy)
                    nc.vector.tensor_copy(adsb1[:, w, :], ps[:, 132:136])
                ci, ro = wslot(wb)
                nc.sync.dma_start(
                    t1s[ci][ro:ro + nw * 128, 0:136]
                    .rearrange("(w p) c -> p w c", w=nw), rows[:, 0:nw, :])
                wb += nw
                if wb == W0:
                    ag_chunk(t1s[0], t1f[0])
                elif wb == W:
                    ag_chunk(t1s[1], t1f[1])

            # ================= edge phase (shared for both layers) ==========
            def edge_phase(layer):
                if layer == 1:
                    tabs, row, heads, ch = t1f, ROW1, H1, C1
                    adsb = adsb1
                    a_s_off = 128
                    rcols = HC1 + H1  # 132
                else:
                    tabs, row, heads, ch = t2f, ROW2, H2, C2
                    adsb = adsb2
                    a_s_off = 64
                    rcols = C2 + H2  # 65
                hc = heads * ch
                nh = heads  # scalar cols per tile
                NIX = (NLO + NHI) // 16
                for g in range(G):
                    w0 = g * gw
                    # --- gather (merged idx load: [lo | hi] per group) ---
                    Gt = gpool.tile([128, T, row], f16, tag=f"G{layer}")
                    ix = ipool.tile([128, NIX], i16, tag="ix")
                    nc.sync.dma_start(ix[:], idx_d[:, g * NIX:(g + 1) * NIX])
                    for off in range(0, NLO, GCAP):
                        sz = min(GCAP, NLO - off)
                        nc.gpsimd.dma_gather(
                            out_ap=Gt[:, off // 128:(off + sz) // 128, :],
                            in_ap=tabs[0][:],
                            idxs_ap=ix[:, off // 16:(off + sz) // 16],
                            num_idxs=sz, num_idxs_reg=sz,
                            elem_size=row)
                    for off in range(0, NHI, GCAP):
                        sz = min(GCAP, NHI - off)
                        nc.gpsimd.dma_gather(
                            out_ap=Gt[:, gw * k_lo + off // 128:gw * k_lo + (off + sz) // 128, :],
                            in_ap=tabs[1][:],
                            idxs_ap=ix[:, (NLO + off) // 16:(NLO + off + sz) // 16],
                            num_idxs=sz, num_idxs_reg=sz,
                            elem_size=row)
                    # --- one-hot: layer 1 streams [e1 | sm]; layer 2 streams
                    # sm only and rebuilds e1 on DVE (u8 is_eq -> f8) ---
                    if layer == 1:
                        es = opool.tile([128, 2 * T, 128], f8, tag="es")
                        nc.sync.dma_start(
                            es[:].rearrange("p t j -> p (t j)"),
                            es_d[:, g * 2 * T * 128:(g + 1) * 2 * T * 128])
                        e1v = es
                        smo = T
                    else:
                        es = opool2.tile([128, T, 128], f8, tag="es2")
                        nc.sync.dma_start(
                            es[:].rearrange("p t j -> p (t j)"),
                            es_d[:, (g * 2 + 1) * T * 128:(g * 2 + 2) * T * 128])
                        dc_sb = ipool.tile([128, T], u8, tag="dc")
                        nc.sync.dma_start(dc_sb[:], dc_d[:, g * T:(g + 1) * T])
                        e1v = opool2.tile([128, T, 128], f8, tag="e1b")
                        nc.vector.tensor_tensor(
                            e1v[:], dc_sb[:].broadcast_to([128, T, 128]),
                            iota_r[:].rearrange("p (one x) -> p one x", one=1)
                                     .broadcast_to([128, T, 128]),
                            OP.is_equal)
                        smo = 0

                    # --- a_d broadcast matmuls ---
                    pad = psad.tile([128, T * nh], f32, tag="pad")
                    for t in range(T):
                        w = w0 + (t // k_lo if t < gw * k_lo else (t - gw * k_lo) // k_hi)
                        nc.tensor.matmul(pad[:, t * nh:(t + 1) * nh],
                                         lhsT=es[:, smo + t, :], rhs=adsb[:, w, :],
                                         start=True, stop=True)
                    # --- per-edge scalars: z = a_s + a_d; p = exp(lrelu(z)) ---
                    z = spool.tile([128, T, nh], f32, tag="z")
                    nc.vector.tensor_add(z[:], Gt[:, :, a_s_off:a_s_off + nh],
                                         pad[:].rearrange("p (t h) -> p t h", h=nh))
                    zl = spool.tile([128, T, nh], f32, tag="zl")
                    nc.scalar.activation(zl[:], z[:], AF.Prelu, alpha=NEG_SLOPE)
                    # psc expanded across channels (ACT broadcast-read), f16
                    pexp = spool.tile([128, T, hc], f16, tag="pexp")
                    nc.scalar.activation(
                        pexp[:].rearrange("p t (h c) -> p t h c", h=nh),
                        zl[:].rearrange("p t (h one) -> p t h one", one=1)
                             .broadcast_to([128, T, nh, ch]),
                        AF.Exp)
                    # --- R = [p*h | p] ---
                    R = rpool.tile([128, T, rcols], f16, tag="R")
                    nc.vector.tensor_mul(R[:, :, 0:hc], Gt[:, :, 0:hc], pexp[:])
                    nc.scalar.activation(R[:, :, hc:hc + nh], zl[:], AF.Exp)
                    # --- segment-sum matmuls ---
                    pw = [psW.tile([128, rcols], f32, tag="psW", name=f"pw{layer}_{g}_{wi}")[:]
                          for wi in range(gw)]
                    for t in range(T):
                        if t < gw * k_lo:
                            wi, first = divmod(t, k_lo)
                            is_first = first == 0
                            is_last = (first == k_lo - 1) and k_hi == 0
                        else:
                            wi, r = divmod(t - gw * k_lo, k_hi)
                            is_first = False
                            is_last = r == k_hi - 1
                        nc.tensor.matmul(pw[wi], lhsT=e1v[:, t, :], rhs=R[:, t, :],
                                         start=is_first, stop=is_last)
                    # --- epilogue per window (DMAs batched per group) ---
                    if layer == 1:
                        rows2 = rowpool.tile([128, gw, 66], f16, tag="rows2")
                    else:
                        og = epool.tile([128, gw, hc], f32, tag="og")
                    for wi in range(gw):
                        w = w0 + wi
                        den = epool.tile([128, nh], f32, tag="den")
                        nc.vector.tensor_scalar_add(den[:], pw[wi][:, hc:hc + nh], EPS)
                        rec = epool.tile([128, nh], f32, tag="rec")
                        nc.vector.reciprocal(rec[:], den[:])
                        if layer == 1:
                            o = epool.tile([128, hc], f32, tag="o")
                            nc.vector.tensor_mul(
                                o[:].rearrange("p (h c) -> p h c", h=heads),
                                pw[wi][:, 0:hc].rearrange("p (h c) -> p h c", h=heads),
                                rec[:].broadcast_to([128, heads, ch]))
                            nc.vector.tensor_add(o[:], o[:], b1_sb[:])
                            # ELU: relu(o) + exp(min(o,0)) - 1
                            r_ = epool.tile([128, hc], f32, tag="relu")
                            nc.scalar.activation(r_[:], o[:], AF.Relu)
                            m_ = epool.tile([128, hc], f32, tag="mneg")
                            nc.vector.tensor_scalar_min(m_[:], o[:], 0.0)
                            nc.scalar.activation(m_[:], m_[:], AF.Exp)
                            act = epool.tile([128, hc], f16, tag="act")
                            nc.vector.scalar_tensor_tensor(
                                act[:], r_[:], -1.0, m_[:], OP.add, OP.add)
                            # transpose into x2T; layer-2 node matmul + table row
                            psT = psTp.tile([128, 128], f16, tag="psT")
                            nc.tensor.transpose(psT[:], act[:], ident[:])
                            nc.vector.tensor_copy(x2T[:, w * 128:(w + 1) * 128], psT[:])
                            ps2 = ps2p.tile([128, 66], f32, tag="ps2")
                            nc.tensor.matmul(ps2[:], lhsT=x2T[:, w * 128:(w + 1) * 128],
                                             rhs=w2e_f16[:], start=True, stop=True)
                            nc.scalar.activation(rows2[:, wi, :], ps2[:], AF.Copy)
                            nc.vector.tensor_copy(adsb2[:, w, :], ps2[:, 65:66])
                        else:
                            nc.vector.scalar_tensor_tensor(
                                og[:, wi, :], pw[wi][:, 0:hc], rec[:], b2_sb[:],
                                OP.mult, OP.add)
                    if layer == 1:
                        ci, ro = wslot(w0)
                        nc.sync.dma_start(
                            t2s[ci][ro:ro + gw * 128, 0:66]
                            .rearrange("(w p) c -> p w c", w=gw), rows2[:])
                        if w0 + gw == W0:
                            ag_chunk(t2s[0], t2f[0])
                        elif w0 + gw == W:
                            ag_chunk(t2s[1], t2f[1])
                    else:
                        nc.sync.dma_start(
                            out_d[w0 * 128:(w0 + gw) * 128, :]
                            .rearrange("(w p) c -> p w c", w=gw), og[:])

            edge_phase(1)
            edge_phase(2)

    nc.compile()
    return nc


# ---------------------------------------------------------------------------
# Entry point
# ---------------------------------------------------------------------------

_CACHE = {}


def _prepare(inputs):
    x = np.ascontiguousarray(np.asarray(inputs["x"], np.float32))
    ei = np.asarray(inputs["edge_index"])
    n_nodes = x.shape[0]
    return pick_config(x, ei, n_nodes)


def _weights_ext(inputs):
    W1 = np.asarray(inputs["W1"], np.float32)
    as1 = np.asarray(inputs["att_src1"], np.float32)
    ad1 = np.asarray(inputs["att_dst1"], np.float32)
    W2 = np.asarray(inputs["W2"], np.float32)
    as2 = np.asarray(inputs["att_src2"], np.float32)
    ad2 = np.asarray(inputs["att_dst2"], np.float32)
    As = np.zeros((HC1, H1), np.float32)
    Ad = np.zeros((HC1, H1), np.float32)
    for h in range(H1):
        As[h * C1:(h + 1) * C1, h] = as1[0, h]
        Ad[h * C1:(h + 1) * C1, h] = ad1[0, h]
    w1e = np.concatenate([W1, W1 @ As, W1 @ Ad], axis=1)           # [128,136]
    w2e = np.concatenate([W2, W2 @ as2[0].T, W2 @ ad2[0].T], axis=1)  # [128,66]
    return np.ascontiguousarray(w1e), np.ascontiguousarray(w2e)


def kernel(**inputs):
    from concourse.bass_utils import run_bass_kernel_spmd

    prep = _prepare(inputs)
    key = (prep["W"], prep["W0"], prep["k_lo"], prep["k_hi"], prep["gw"])
    if key not in _CACHE:
        _CACHE[key] = build_program(dict(
            W=prep["W"], W0=prep["W0"], P=prep["P"], K=prep["K"],
            k_lo=prep["k_lo"], k_hi=prep["k_hi"], gw=prep["gw"]))
    nc = _CACHE[key]

    in_maps = build_in_maps(inputs, prep)
    res = run_bass_kernel_spmd(nc, in_maps, core_ids=list(range(NCORES)))
    return assemble_output(res.results, prep)


def build_in_maps(inputs, prep):
    import concourse.mybir as mybir
    f8np = mybir.dt.np(mybir.dt.float8e4)
    x = np.ascontiguousarray(np.asarray(inputs["x"], np.float32))
    b1 = np.tile(np.asarray(inputs["b1"], np.float32).reshape(1, HC1), (128, 1))
    b2 = np.tile(np.asarray(inputs["b2"], np.float32).reshape(1, C2), (128, 1))
    w1e, w2e = _weights_ext(inputs)
    n_nodes, P, W = prep["n_nodes"], prep["P"], prep["W"]
    po = prep["po"]
    ident = np.eye(128, dtype=np.float16)
    in_maps = []
    # xT_pad per core: columns = core-local padded slots (window-major)
    xT_all = np.zeros((NCORES, IN_CH, P), np.float32)
    node_ids = np.arange(n_nodes)
    c_of = po // P
    col = po % P
    xT_all[c_of, :, col] = x[node_ids]  # fancy: sets [ch] vectors
    for c in range(NCORES):
        m = prep["meta"][c]
        im = dict(
            xT=np.ascontiguousarray(xT_all[c]),
            w1e=w1e, w2e=w2e,
            idx=np.ascontiguousarray(m["idx"]),
            es=m["es"].view(f8np),
            dc=m["dc"],
            iota_r=np.tile(np.arange(128, dtype=np.uint8), (128, 1)),
            ident=ident,
            b1=b1, b2=b2,
        )
        in_maps.append(im)
    return in_maps


def assemble_output(results, prep):
    full = np.concatenate([results[c]["out"] for c in range(NCORES)], axis=0)
    return np.ascontiguousarray(full[prep["po"]]).astype(np.float32)
